# revision 1
# baseline (speedup 1.0000x reference)
"""Trainium2 Bass kernel for nn_DeliveryEventEncoder.

Strategy: pure data parallel across 8 NeuronCores (4 buildings = 128 units
per core). Activations are kept in feature-major layout [feat(128 part),
seq(256 free)] so every weight matmul streams 256 columns; matmul inputs
are bf16 (1 cyc/row on PE), accumulation is fp32 in PSUM, LayerNorm
stats/softmax denominators are fp32. The ragged key mask folds into v and
the softmax denominator (no masking of exp tiles); the query mask folds
into LN2's rstd so the ragged sum-pool is a plain ones-matmul.

The per-unit work is emitted in two phases per group of 8 units: phase A
(everything through softmax exp — act-func-set "exp") for all 8 units,
then phase B (LayerNorm sqrt, relu, copies — act-func-set "sqrt") for all
8. The ACT PWP table reload costs 1.28us, so alternating exp/sqrt per
unit would burn ~330us/core; grouping drops it to 2 reloads per 8 units.
"""

import os
import numpy as np
import ml_dtypes

import concourse.bass as bass
import concourse.bacc as bacc_mod
import concourse.mybir as mybir
import concourse.tile as tile
from concourse.bass_utils import run_bass_kernel_spmd
from concourse.masks import make_identity

F32 = mybir.dt.float32
BF16 = mybir.dt.bfloat16
AF = mybir.ActivationFunctionType
ALU = mybir.AluOpType
NPBF = ml_dtypes.bfloat16

B, U, L, DSEQ, H, DOUT = 32, 32, 256, 5, 128, 128
TODV, TODD, AGGD, UNITD = 5, 3, 7, 16
NCORES = 8
BPC = B // NCORES          # buildings per core
NU = BPC * U               # units per core (128)
GRP = int(os.environ.get('KGRP', '8'))   # units per X-group DMA / phase block
NGRP = NU // GRP
CSCALE = 1.0 / np.sqrt(H)
EPS = 1e-5


def build_nc(wts):
    """Build the SPMD Bass module. `wts`: numpy bf16 weight arrays (already
    transposed for lhsT use), baked in as inline consts."""
    nc = bacc_mod.Bacc()

    x_in = nc.dram_tensor("xg", [NGRP, DSEQ, GRP * L], BF16, kind="ExternalInput")
    m01_in = nc.dram_tensor("m01", [128, NU * 2], F32, kind="ExternalInput")
    m01b_in = nc.dram_tensor("m01b", [128, NU * 2], BF16, kind="ExternalInput")
    s_in = nc.dram_tensor("S", [NU, BPC], BF16, kind="ExternalInput")
    tail_in = nc.dram_tensor("tail", [AGGD + TODD, BPC], BF16, kind="ExternalInput")
    out_t = nc.dram_tensor("outT", [DOUT, BPC], F32, kind="ExternalOutput")

    dW = {k: nc.inline_tensor(v, name=k) for k, v in wts.items()}

    cfg = dict(xp=2, wk=3, nt=3, sm=8, pp=2 * GRP + 1, ps=3, pn=3, pc=1, pa=1)
    for kv in os.environ.get("KPOOLS", "").split(","):
        if kv:
            k_, v_ = kv.split("=")
            cfg[k_] = int(v_)

    with tile.TileContext(nc) as tc:
        with (
            tc.tile_pool(name="singles", bufs=1) as singles,
            tc.tile_pool(name="xpool", bufs=cfg["xp"]) as xpool,
            tc.tile_pool(name="work", bufs=cfg["wk"]) as work,
            tc.tile_pool(name="nat", bufs=cfg["nt"]) as natp,
            tc.tile_pool(name="small", bufs=cfg["sm"]) as small,
            tc.tile_pool(name="pipe", bufs=cfg["pp"]) as pipe,
            tc.tile_pool(name="pipe2", bufs=2 * cfg["pp"]) as pipe2,
            tc.tile_pool(name="ps", bufs=cfg["ps"], space="PSUM") as ps,
            tc.tile_pool(name="psn", bufs=cfg["pn"], space="PSUM") as psn,
            tc.tile_pool(name="pcol", bufs=cfg["pc"], space="PSUM") as pcol,
            tc.tile_pool(name="pacc", bufs=cfg["pa"], space="PSUM") as pacc,
        ):
            # ---- constants into SBUF ----
            def load_w(name, p, f):
                t = singles.tile([p, f], BF16, tag=name)
                nc.gpsimd.dma_start(out=t, in_=dW[name][:, :])
                return t

            w_in = load_w("w_inT", DSEQ, H)
            w_g = load_w("w_gT", H, H)
            w_v = load_w("w_vT", H, H)
            w_o = load_w("w_oT", H, H)
            w_f1 = load_w("w_f1T", H, H)
            w_f2 = load_w("w_f2T", H, H)
            w_u = load_w("w_uT", H, UNITD)
            w_c1 = load_w("w_c1T", UNITD + AGGD + TODD, H)
            w_c2 = load_w("w_c2T", H, DOUT)

            ident = singles.tile([128, 128], F32, tag="ident")
            make_identity(nc, ident)
            ones_b = singles.tile([128, 1], BF16, tag="ones")
            nc.vector.memset(ones_b, 1.0)
            eps_col = singles.tile([128, 1], F32, tag="eps")
            nc.vector.memset(eps_col, EPS)

            s_sb = singles.tile([NU, BPC], BF16, tag="S")
            nc.gpsimd.dma_start(out=s_sb, in_=s_in[:, :])
            m01_all = singles.tile([128, NU * 2], F32, tag="m01")
            nc.gpsimd.dma_start(out=m01_all, in_=m01_in[:, :])
            m01b = singles.tile([128, NU * 2], BF16, tag="m01b")
            nc.gpsimd.dma_start(out=m01b, in_=m01b_in[:, :])

            pooled = singles.tile([H, NU], BF16, tag="pooled")

            def phase_a(xs, kk, u):
                """emb/q/k/v/scores/exp for one unit (act set: exp)."""
                xu = xs[:, kk * L:(kk + 1) * L]

                emb_ps = ps.tile([H, L], F32, tag="ps")
                nc.tensor.matmul(emb_ps, w_in, xu, start=True, stop=True)
                embT = work.tile([H, L], BF16, tag="embT")
                (nc.vector if os.environ.get("KCPE") else nc.any).tensor_copy(embT, emb_ps)

                embn = []
                for lt in range(2):
                    en_ps = psn.tile([128, H], F32, tag="psn")
                    nc.tensor.matmul(
                        en_ps, xu[:, lt * 128:(lt + 1) * 128], w_in,
                        start=True, stop=True)
                    en = pipe2.tile([128, H], F32, tag="embn")
                    nc.any.tensor_copy(en, en_ps)
                    embn.append(en)

                y_ps = ps.tile([H, L], F32, tag="ps")
                nc.tensor.matmul(y_ps, w_g, embT, start=True, stop=True)
                yT = work.tile([H, L], BF16, tag="yT")
                (nc.vector if os.environ.get("KCPE") else nc.any).tensor_copy(yT, y_ps)

                v_s = []
                for mt in range(2):
                    v_ps = psn.tile([128, H], F32, tag="psn")
                    nc.tensor.matmul(
                        v_ps, embT[:, mt * 128:(mt + 1) * 128], w_v,
                        start=True, stop=True)
                    vs = pipe.tile([128, H], BF16, tag=f"v{mt}")
                    # key mask folds into v (per-partition scale)
                    if os.environ.get("KVMASK") == "dve":
                        nc.vector.tensor_scalar_mul(
                            out=vs, in0=v_ps,
                            scalar1=m01_all[:, 2 * u + mt:2 * u + mt + 1])
                    else:
                        nc.scalar.activation(
                            out=vs, in_=v_ps, func=AF.Copy, bias=0.0,
                            scale=m01_all[:, 2 * u + mt:2 * u + mt + 1])
                    v_s.append(vs)

                exp_s = []
                for mt in range(2):
                    sc_ps = ps.tile([128, L], F32, tag="ps")
                    nc.tensor.matmul(
                        sc_ps, embT[:, mt * 128:(mt + 1) * 128], yT,
                        start=True, stop=True)
                    es = pipe.tile([128, L], BF16, tag=f"exp{mt}")
                    nc.scalar.activation(
                        out=es, in_=sc_ps, func=AF.Exp, bias=0.0, scale=CSCALE)
                    exp_s.append(es)
                return dict(u=u, embn=embn, v_s=v_s, exp_s=exp_s)

            def phase_b(st):
                """attention apply + LNs + FFN + pool (act set: sqrt)."""
                u, embn, v_s, exp_s = st["u"], st["embn"], st["v_s"], st["exp_s"]

                rec = []
                for lt in range(2):
                    den_ps = pcol.tile([128, 1], F32, tag="pcol")
                    for mt in range(2):
                        nc.tensor.matmul(
                            den_ps, exp_s[mt][:, lt * 128:(lt + 1) * 128],
                            m01b[:, 2 * u + mt:2 * u + mt + 1],
                            start=(mt == 0), stop=(mt == 1))
                    rc = small.tile([128, 1], F32, tag="rec")
                    nc.vector.reciprocal(rc, den_ps)
                    rec.append(rc)

                ao_ps = ps.tile([H, L], F32, tag="ps")
                for mt in range(2):
                    nc.tensor.matmul(ao_ps, v_s[mt], exp_s[mt],
                                     start=(mt == 0), stop=(mt == 1))
                aoT = work.tile([H, L], BF16, tag="aoT")
                nc.any.tensor_copy(aoT, ao_ps)

                x1_nat = []
                for lt in range(2):
                    sl = slice(lt * 128, (lt + 1) * 128)
                    pon_ps = psn.tile([128, H], F32, tag="psn")
                    nc.tensor.matmul(pon_ps, aoT[:, sl], w_o,
                                     start=True, stop=True)
                    x1in = natp.tile([128, H], F32, tag="x1in")
                    s1 = small.tile([128, 1], F32, tag="s1")
                    nc.vector.scalar_tensor_tensor(
                        out=x1in, in0=pon_ps, scalar=rec[lt], in1=embn[lt],
                        op0=ALU.mult, op1=ALU.add, accum_out=s1)
                    sq = natp.tile([128, H], BF16, tag="sq")
                    q1 = small.tile([128, 1], F32, tag="q1")
                    nc.scalar.activation(out=sq, in_=x1in, func=AF.Square,
                                         bias=0.0, scale=1.0, accum_out=q1)
                    mean = small.tile([128, 1], F32, tag="mean")
                    nc.vector.tensor_scalar(
                        out=mean, in0=s1, scalar1=1.0 / H, scalar2=None,
                        op0=ALU.mult)
                    msq = small.tile([128, 1], F32, tag="msq")
                    nc.vector.tensor_tensor(
                        out=msq, in0=mean, in1=mean, op=ALU.mult)
                    var = small.tile([128, 1], F32, tag="var")
                    nc.vector.scalar_tensor_tensor(
                        out=var, in0=q1, scalar=1.0 / H, in1=msq,
                        op0=ALU.mult, op1=ALU.subtract)
                    sd = small.tile([128, 1], F32, tag="sd")
                    nc.scalar.activation(out=sd, in_=var, func=AF.Sqrt,
                                         bias=eps_col, scale=1.0)
                    rs = small.tile([128, 1], F32, tag="rs")
                    nc.vector.reciprocal(rs, sd)
                    x1 = natp.tile([128, H], F32, tag="x1")
                    nc.vector.tensor_scalar(
                        out=x1, in0=x1in, scalar1=mean, scalar2=rs,
                        op0=ALU.subtract, op1=ALU.mult)
                    x1_nat.append(x1)

                x1T = work.tile([H, L], BF16, tag="x1T")
                for lt in range(2):
                    x1t_ps = psn.tile([128, H], F32, tag="psn")
                    nc.tensor.transpose(x1t_ps, x1_nat[lt], ident)
                    nc.any.tensor_copy(x1T[:, lt * 128:(lt + 1) * 128], x1t_ps)

                f1_ps = ps.tile([H, L], F32, tag="ps")
                nc.tensor.matmul(f1_ps, w_f1, x1T, start=True, stop=True)
                f1 = work.tile([H, L], BF16, tag="f1")
                nc.scalar.activation(out=f1, in_=f1_ps, func=AF.Relu,
                                     bias=0.0, scale=1.0)

                pool_ps = pacc.tile([H, 1], F32, tag="pacc")
                for lt in range(2):
                    sl = slice(lt * 128, (lt + 1) * 128)
                    f2n_ps = psn.tile([128, H], F32, tag="psn")
                    nc.tensor.matmul(f2n_ps, f1[:, sl], w_f2,
                                     start=True, stop=True)
                    x2in = natp.tile([128, H], F32, tag="x2in")
                    s2 = small.tile([128, 1], F32, tag="s1")
                    nc.vector.scalar_tensor_tensor(
                        out=x2in, in0=f2n_ps, scalar=1.0, in1=x1_nat[lt],
                        op0=ALU.mult, op1=ALU.add, accum_out=s2)
                    sq2 = natp.tile([128, H], BF16, tag="sq")
                    q2 = small.tile([128, 1], F32, tag="q1")
                    nc.scalar.activation(out=sq2, in_=x2in, func=AF.Square,
                                         bias=0.0, scale=1.0, accum_out=q2)
                    mean2 = small.tile([128, 1], F32, tag="mean")
                    nc.vector.tensor_scalar(
                        out=mean2, in0=s2, scalar1=1.0 / H, scalar2=None,
                        op0=ALU.mult)
                    msq2 = small.tile([128, 1], F32, tag="msq")
                    nc.vector.tensor_tensor(
                        out=msq2, in0=mean2, in1=mean2, op=ALU.mult)
                    var2 = small.tile([128, 1], F32, tag="var")
                    nc.vector.scalar_tensor_tensor(
                        out=var2, in0=q2, scalar=1.0 / H, in1=msq2,
                        op0=ALU.mult, op1=ALU.subtract)
                    sd2 = small.tile([128, 1], F32, tag="sd")
                    nc.scalar.activation(out=sd2, in_=var2, func=AF.Sqrt,
                                         bias=eps_col, scale=1.0)
                    rs2 = small.tile([128, 1], F32, tag="rs")
                    nc.vector.reciprocal(rs2, sd2)
                    rs2m = small.tile([128, 1], F32, tag="rs2m")
                    nc.vector.tensor_scalar(
                        out=rs2m, in0=rs2,
                        scalar1=m01_all[:, 2 * u + lt:2 * u + lt + 1],
                        scalar2=None, op0=ALU.mult)
                    x2 = natp.tile([128, H], BF16, tag="x2")
                    nc.vector.tensor_scalar(
                        out=x2, in0=x2in, scalar1=mean2, scalar2=rs2m,
                        op0=ALU.subtract, op1=ALU.mult)
                    nc.tensor.matmul(pool_ps, x2, ones_b,
                                     start=(lt == 0), stop=(lt == 1))
                nc.any.tensor_copy(pooled[:, u:u + 1], pool_ps)

            # ---- per-group two-phase emission ----
            for g in range(NGRP):
                xs = xpool.tile([DSEQ, GRP * L], BF16, tag="X")
                nc.sync.dma_start(out=xs, in_=x_in[g, :, :])
                states = [phase_a(xs, kk, g * GRP + kk) for kk in range(GRP)]
                for st in states:
                    phase_b(st)

            # ---- per-core tail: unit_fc, building-sum, fusion MLP ----
            u16_ps = psn.tile([UNITD, NU], F32, tag="psn")
            nc.tensor.matmul(u16_ps, w_u, pooled, start=True, stop=True)
            u16 = work.tile([UNITD, NU], F32, tag="u16")
            nc.scalar.activation(out=u16, in_=u16_ps, func=AF.Relu,
                                 bias=0.0, scale=1.0)

            u16t_ps = psn.tile([NU, UNITD], F32, tag="psn")
            nc.tensor.transpose(u16t_ps, u16, ident[:UNITD, :UNITD])
            u16t = work.tile([NU, UNITD], BF16, tag="u16t")
            nc.any.tensor_copy(u16t, u16t_ps)

            seq_ps = psn.tile([UNITD, BPC], F32, tag="psn")
            nc.tensor.matmul(seq_ps, u16t, s_sb, start=True, stop=True)

            fused = work.tile([UNITD + AGGD + TODD, BPC], BF16, tag="fused")
            nc.any.tensor_copy(fused[:UNITD, :], seq_ps)
            nc.gpsimd.dma_start(out=fused[UNITD:, :], in_=tail_in[:, :])

            h1_ps = psn.tile([H, BPC], F32, tag="psn")
            nc.tensor.matmul(h1_ps, w_c1, fused, start=True, stop=True)
            h1 = work.tile([H, BPC], BF16, tag="h1")
            nc.scalar.activation(out=h1, in_=h1_ps, func=AF.Relu,
                                 bias=0.0, scale=1.0)

            o_ps = psn.tile([DOUT, BPC], F32, tag="psn")
            nc.tensor.matmul(o_ps, w_c2, h1, start=True, stop=True)
            o_s = work.tile([DOUT, BPC], F32, tag="osb")
            nc.scalar.activation(out=o_s, in_=o_ps, func=AF.Relu,
                                 bias=0.0, scale=1.0)
            nc.sync.dma_start(out=out_t[:, :], in_=o_s)

    return nc


def _prep_weights(inputs):
    ipw = np.asarray(inputs["in_proj_w"])
    wts = {
        "w_inT": np.asarray(inputs["W_in"]).T,       # [5,128]
        "w_gT": (ipw[0:H] @ ipw[H:2 * H].T),          # Wq^T Wk composed [128,128]
        "w_vT": ipw[2 * H:3 * H].T,
        "w_oT": np.asarray(inputs["out_proj_w"]).T,
        "w_f1T": np.asarray(inputs["W_ff1"]).T,
        "w_f2T": np.asarray(inputs["W_ff2"]).T,
        "w_uT": np.asarray(inputs["W_unit"]).T,       # [128,16]
        "w_c1T": np.asarray(inputs["W_fc1"]).T,       # [26,128]
        "w_c2T": np.asarray(inputs["W_fc2"]).T,       # [128,128]
    }
    wts = {k: np.ascontiguousarray(v.astype(NPBF)) for k, v in wts.items()}
    # the kernel folds no biases / LN affines: assert they are trivial
    for nm in ("b_in", "in_proj_b", "out_proj_b", "b_ff1", "b_ff2",
               "ln1_b", "ln2_b", "b_unit", "b_fc1", "b_fc2"):
        assert np.max(np.abs(np.asarray(inputs[nm]))) == 0.0, f"{nm} nonzero"
    for nm in ("ln1_w", "ln2_w"):
        assert np.allclose(np.asarray(inputs[nm]), 1.0), f"{nm} nontrivial"
    return wts


def make_in_maps(inputs):
    x_seq = np.asarray(inputs["x_seq"], dtype=np.float32)       # [B,U,L,5]
    lengths = np.asarray(inputs["lengths"])                      # [B,U] int
    x_agg = np.asarray(inputs["x_agg_quant"], dtype=np.float32)  # [B,7]
    tod_emb = np.asarray(inputs["tod_emb"], dtype=np.float32)    # [5,3]
    tod_idx = np.asarray(inputs["tod_idx"])                      # [B] int

    in_maps = []
    for c in range(NCORES):
        bs = slice(c * BPC, (c + 1) * BPC)
        xc = x_seq[bs].reshape(NU, L, DSEQ).transpose(0, 2, 1)   # [128,5,256]
        xg = np.ascontiguousarray(
            xc.reshape(NGRP, GRP, DSEQ, L).transpose(0, 2, 1, 3)
            .reshape(NGRP, DSEQ, GRP * L)).astype(NPBF)
        lens = lengths[bs].reshape(NU).astype(np.float32)
        iota = np.arange(L, dtype=np.float32).reshape(2, 128).T  # [128p, 2 tiles]
        # resident mask tile [128p, NU*2]: col 2u+t = (p + 128t) < len[u]
        m01 = (iota[:, None, :] < lens[None, :, None]).astype(np.float32)
        m01 = m01.reshape(128, NU * 2)
        S = np.zeros((NU, BPC), np.float32)
        S[np.arange(NU), np.arange(NU) // U] = 1.0
        tail = np.concatenate(
            [x_agg[bs].T, tod_emb[tod_idx[bs]].T], axis=0)
        in_maps.append({"xg": xg, "m01": np.ascontiguousarray(m01),
                        "m01b": np.ascontiguousarray(m01).astype(NPBF),
                        "S": S.astype(NPBF),
                        "tail": np.ascontiguousarray(tail).astype(NPBF)})
    return in_maps


def kernel(_trace=False, **inputs):
    wts = _prep_weights(inputs)
    nc = build_nc(wts)
    if not nc.is_finalized():
        nc.finalize()
    in_maps = make_in_maps(inputs)
    res = run_bass_kernel_spmd(nc, in_maps, core_ids=list(range(NCORES)),
                               trace=_trace)
    out = np.zeros((B, DOUT), np.float32)
    for c in range(NCORES):
        out[c * BPC:(c + 1) * BPC, :] = res.results[c]["outT"].T
    if _trace:
        kernel._last_results = res
    return out



# revision 11
# speedup vs baseline: 1.2582x; 1.2582x over previous
"""Trainium2 Bass kernel for nn_DeliveryEventEncoder.

Pure data parallel across 8 NeuronCores (4 buildings = 128 units per core).
Activations kept feature-major [feat(128 part), seq(free)]; matmul inputs
bf16, fp32 PSUM accumulation.

Cost-model-driven design (TimelineSim): per-op fixed overheads dominate
(ACT ~185ns, DVE ~60/125ns), so per-unit op count is minimized and spread
across ACT/DVE/Pool:
 - LayerNorm stats: mean via free accum_out on the residual-add op, sum of
   squares via one DVE tensor_tensor_reduce per 128-row chunk; variance /
   sqrt / reciprocal are group-batched over [128, 2*GRP] tiles.
 - Ragged key mask folds into the softmax exp bias (0 / -30 per key row),
   so the v evacuation is a plain copy; query mask folds into LN2's rstd.
 - Per-chunk PSUM evacuations merged into single [128, 256] ops; the
   normalization applies run in the DVE 4x perf mode (bf16, SBUF).
 - GRP=32 units per phase block: 2 ACT table reloads per group (8 total).
 - PSUM is bank-granular (8 x 2KB): tags psA x3, psB x2, natps x2,
   colps x1.
"""

import os
import numpy as np
import ml_dtypes

import concourse.bass as bass
import concourse.bacc as bacc_mod
import concourse.mybir as mybir
import concourse.tile as tile
from concourse.bass_utils import run_bass_kernel_spmd
from concourse.masks import make_identity

F32 = mybir.dt.float32
BF16 = mybir.dt.bfloat16
AF = mybir.ActivationFunctionType
ALU = mybir.AluOpType
NPBF = ml_dtypes.bfloat16

B, U, L, DSEQ, H, DOUT = 32, 32, 256, 5, 128, 128
TODV, TODD, AGGD, UNITD = 5, 3, 7, 16
NCORES = 8
BPC = B // NCORES          # buildings per core
NU = BPC * U               # units per core (128)
GRP = int(os.environ.get('KGRP', '32'))  # units per phase block
NGRP = NU // GRP
MB = int(os.environ.get('KMB', '4'))     # units per den/recip micro-batch
CSCALE = 1.0 / np.sqrt(H)
EPS = 1e-5
NEGB = -30.0               # exp bias for masked keys

# evacuation engine assignment (tunable)
EV = dict(embT='act', yT='pool', vs='pool', aoT='pool', x1T='pool',
          f1='act')
for kv in os.environ.get('KEV', '').split(','):
    if kv:
        k_, v_ = kv.split('=')
        EV[k_] = v_


def build_nc(wts):
    nc = bacc_mod.Bacc()

    x_in = nc.dram_tensor("xg", [NGRP, DSEQ, GRP * L], BF16, kind="ExternalInput")
    m01_in = nc.dram_tensor("m01", [128, NU * 2], F32, kind="ExternalInput")
    eb_in = nc.dram_tensor("eb", [128, NU * 2], F32, kind="ExternalInput")
    s_in = nc.dram_tensor("S", [NU, BPC], BF16, kind="ExternalInput")
    tail_in = nc.dram_tensor("tail", [AGGD + TODD, BPC], BF16, kind="ExternalInput")
    out_t = nc.dram_tensor("outT", [DOUT, BPC], F32, kind="ExternalOutput")

    dW = {k: nc.inline_tensor(v, name=k) for k, v in wts.items()}

    cfg = dict(xp=2, wk=3, sm=8, es=2 * (MB + 1), x12=3, sq=2,
               ln=10, psA=2, psB=2, psT=1, nat=2, col=1)
    for kv in os.environ.get("KPOOLS", "").split(","):
        if kv:
            k_, v_ = kv.split("=")
            cfg[k_] = int(v_)

    def evac(engine, out, in_, relu=False):
        if engine == 'act':
            nc.scalar.activation(out=out, in_=in_,
                                 func=AF.Relu if relu else AF.Copy,
                                 bias=0.0, scale=1.0)
        elif engine == 'dve':
            if relu:
                nc.vector.tensor_scalar(out=out, in0=in_, scalar1=0.0,
                                        scalar2=None, op0=ALU.max)
            else:
                nc.vector.tensor_copy(out, in_)
        else:
            if relu:
                nc.gpsimd.tensor_scalar(out=out, in0=in_, scalar1=0.0,
                                        scalar2=None, op0=ALU.max)
            else:
                nc.gpsimd.tensor_copy(out, in_)

    with tile.TileContext(nc) as tc:
        with (
            tc.tile_pool(name="singles", bufs=1) as singles,
            tc.tile_pool(name="persist", bufs=1) as persist,
            tc.tile_pool(name="xpool", bufs=cfg["xp"]) as xpool,
            tc.tile_pool(name="work", bufs=cfg["wk"]) as work,
            tc.tile_pool(name="small", bufs=cfg["sm"]) as small,
            tc.tile_pool(name="espool", bufs=cfg["es"]) as espool,
            tc.tile_pool(name="x12p", bufs=cfg["x12"]) as x12p,
            tc.tile_pool(name="sqp", bufs=cfg["sq"]) as sqp,
            tc.tile_pool(name="lnp", bufs=cfg["ln"]) as lnp,
            tc.tile_pool(name="psA", bufs=cfg["psA"], space="PSUM") as psA,
            tc.tile_pool(name="psB", bufs=cfg["psB"], space="PSUM") as psB,
            tc.tile_pool(name="psT", bufs=cfg["psT"], space="PSUM") as psT,
            tc.tile_pool(name="natps", bufs=cfg["nat"], space="PSUM") as natps,
            tc.tile_pool(name="colps", bufs=cfg["col"], space="PSUM") as colps,
        ):
            # ---- constants into SBUF ----
            def load_w(name, p, f):
                t = singles.tile([p, f], BF16, tag=name)
                nc.gpsimd.dma_start(out=t, in_=dW[name][:, :])
                return t

            w_in = load_w("w_inT", DSEQ, H)
            w_g = load_w("w_gT", H, H)
            w_v = load_w("w_vT", H, H)
            w_o = load_w("w_oT", H, H)
            w_f1 = load_w("w_f1T", H, H)
            w_f2 = load_w("w_f2T", H, H)
            w_u = load_w("w_uT", H, UNITD)
            w_c1 = load_w("w_c1T", UNITD + AGGD + TODD, H)
            w_c2 = load_w("w_c2T", H, DOUT)

            ident = singles.tile([128, 128], F32, tag="ident")
            make_identity(nc, ident)
            ident_b = singles.tile([128, 128], BF16, tag="identb")
            nc.vector.tensor_copy(ident_b, ident)
            ones_b = singles.tile([128, 1], BF16, tag="ones")
            nc.vector.memset(ones_b, 1.0)
            eps_col = singles.tile([128, 1], F32, tag="eps")
            nc.vector.memset(eps_col, EPS * H * H)

            s_sb = singles.tile([NU, BPC], BF16, tag="S")
            nc.gpsimd.dma_start(out=s_sb, in_=s_in[:, :])
            m01_all = singles.tile([128, NU * 2], F32, tag="m01")
            nc.gpsimd.dma_start(out=m01_all, in_=m01_in[:, :])
            eb_all = singles.tile([128, NU * 2], F32, tag="eb")
            nc.gpsimd.dma_start(out=eb_all, in_=eb_in[:, :])

            pooled = singles.tile([H, NU], BF16, tag="pooled")

            def phase_a(xs, kk, u):
                """emb/y/v/scores/exp + den matmuls for one unit."""
                xu = xs[:, kk * L:(kk + 1) * L]

                emb_ps = psA.tile([H, L], F32, tag="psA")
                nc.tensor.matmul(emb_ps, w_in, xu, start=True, stop=True)
                embT = work.tile([H, L], BF16, tag="embT")
                evac(EV['embT'], embT, emb_ps)

                y_ps = psA.tile([H, L], F32, tag="psA")
                nc.tensor.matmul(y_ps, w_g, embT, start=True, stop=True)
                yT = work.tile([H, L], BF16, tag="yT")
                evac(EV['yT'], yT, y_ps)

                v_ps = psA.tile([H, L], F32, tag="psA")
                for mt in range(2):
                    nc.tensor.matmul(
                        v_ps[:, mt * H:(mt + 1) * H],
                        embT[:, mt * 128:(mt + 1) * 128], w_v,
                        start=True, stop=True)
                vs = work.tile([128, 2 * H], BF16, tag="vs")
                evac(EV['vs'], vs, v_ps)

                exp_s = []
                for mt in range(2):
                    sc_ps = psA.tile([128, L], F32, tag="psA")
                    nc.tensor.matmul(
                        sc_ps, embT[:, mt * 128:(mt + 1) * 128], yT,
                        start=True, stop=True)
                    es = espool.tile([128, L], BF16, tag=f"exp{mt}")
                    nc.scalar.activation(
                        out=es, in_=sc_ps, func=AF.Exp,
                        bias=eb_all[:, 2 * u + mt:2 * u + mt + 1],
                        scale=CSCALE)
                    exp_s.append(es)
                return dict(xu=xu, vs=vs, exp_s=exp_s)

            def den_mm(st, den_g, kk):
                """4 cheap matmuls: den cols 2kk, 2kk+1 of the micro-batch
                PSUM tile."""
                for lt in range(2):
                    col = den_g[:, 2 * kk + lt:2 * kk + lt + 1]
                    for mt in range(2):
                        nc.tensor.matmul(
                            col,
                            st["exp_s"][mt][:, lt * 128:(lt + 1) * 128],
                            ones_b, start=(mt == 0), stop=(mt == 1))

            def phase_b1(st, rec, kk, s1_g, q1_g, x1in, ug):
                """attention out + out_proj + residual + LN1 accumulations.
                kk: micro-batch-local index (rec cols); ug: group-local
                unit index (stat cols)."""
                xu, vs, exp_s = st["xu"], st["vs"], st["exp_s"]

                ao_ps = psB.tile([H, L], F32, tag="psB")
                for mt in range(2):
                    nc.tensor.matmul(ao_ps, vs[:, mt * H:(mt + 1) * H],
                                     exp_s[mt], start=(mt == 0), stop=(mt == 1))
                aoT = work.tile([H, L], BF16, tag="aoT")
                evac(EV['aoT'], aoT, ao_ps)

                # natural emb [l, H] per chunk, recomputed here so the PSUM
                # bank lives only across the residual read
                en_ps = psA.tile([128, 2 * H], F32, tag="psA")
                for lt in range(2):
                    nc.tensor.matmul(
                        en_ps[:, lt * H:(lt + 1) * H],
                        xu[:, lt * 128:(lt + 1) * 128], w_in,
                        start=True, stop=True)
                pon_ps = natps.tile([128, 2 * H], F32, tag="natps")
                for lt in range(2):
                    nc.tensor.matmul(pon_ps[:, lt * H:(lt + 1) * H],
                                     aoT[:, lt * 128:(lt + 1) * 128], w_o,
                                     start=True, stop=True)
                for lt in range(2):
                    sl = slice(lt * H, (lt + 1) * H)
                    nc.vector.scalar_tensor_tensor(
                        out=x1in[:, sl], in0=pon_ps[:, sl],
                        scalar=rec[:, 2 * kk + lt:2 * kk + lt + 1],
                        in1=en_ps[:, sl], op0=ALU.mult, op1=ALU.add,
                        accum_out=s1_g[:, 2 * ug + lt:2 * ug + lt + 1])
                for lt in range(2):
                    sl = slice(lt * H, (lt + 1) * H)
                    scr = sqp.tile([128, H], BF16, tag="scr")
                    nc.vector.tensor_tensor_reduce(
                        out=scr, in0=x1in[:, sl], in1=x1in[:, sl],
                        scale=1.0, scalar=0.0, op0=ALU.mult, op1=ALU.add,
                        accum_out=q1_g[:, 2 * ug + lt:2 * ug + lt + 1])

            def ln_stats(s_g, q_g, cols, mask_cols=None):
                """Batched LN stats on [128, cols]: mean = s/H and
                rstd = H / sqrt(H*q - s^2 + H^2 eps) (times mask if given)."""
                mean = lnp.tile([128, cols], F32, tag="mean")
                nc.vector.tensor_scalar(out=mean, in0=s_g, scalar1=1.0 / H,
                                        scalar2=None, op0=ALU.mult)
                sq = lnp.tile([128, cols], F32, tag="sq")
                nc.vector.tensor_tensor(out=sq, in0=s_g, in1=s_g, op=ALU.mult)
                var = lnp.tile([128, cols], F32, tag="var")
                nc.vector.scalar_tensor_tensor(
                    out=var, in0=q_g, scalar=float(H), in1=sq,
                    op0=ALU.mult, op1=ALU.subtract)
                sd = lnp.tile([128, cols], F32, tag="sd")
                nc.scalar.activation(out=sd, in_=var, func=AF.Sqrt,
                                     bias=eps_col, scale=1.0)
                rstd = lnp.tile([128, cols], F32, tag="rstd")
                nc.vector.reciprocal(rstd, sd)
                rstdm = lnp.tile([128, cols], F32, tag="rstdm")
                if mask_cols is not None:
                    nc.vector.scalar_tensor_tensor(
                        out=rstdm, in0=rstd, scalar=float(H), in1=mask_cols,
                        op0=ALU.mult, op1=ALU.mult)
                else:
                    nc.vector.tensor_scalar(out=rstdm, in0=rstd,
                                            scalar1=float(H), scalar2=None,
                                            op0=ALU.mult)
                return mean, rstdm

            def phase_b2(x1in, mean1, rstd1, kk, s2_g, q2_g, x2in):
                """LN1 apply, transpose, FFN, residual, LN2 accumulations."""
                x1 = x12p.tile([128, 2 * H], BF16, tag="x1")
                for lt in range(2):
                    sl = slice(lt * H, (lt + 1) * H)
                    nc.vector.tensor_scalar(
                        out=x1[:, sl], in0=x1in[:, sl],
                        scalar1=mean1[:, 2 * kk + lt:2 * kk + lt + 1],
                        scalar2=rstd1[:, 2 * kk + lt:2 * kk + lt + 1],
                        op0=ALU.subtract, op1=ALU.mult)

                x1t_ps = psT.tile([H, L], BF16, tag="psT")
                for lt in range(2):
                    nc.tensor.transpose(
                        x1t_ps[:, lt * 128:(lt + 1) * 128],
                        x1[:, lt * H:(lt + 1) * H], ident_b)
                x1T = work.tile([H, L], BF16, tag="x1T")
                evac(EV['x1T'], x1T, x1t_ps)

                f1_ps = psB.tile([H, L], F32, tag="psB")
                nc.tensor.matmul(f1_ps, w_f1, x1T, start=True, stop=True)
                f1 = work.tile([H, L], BF16, tag="f1")
                evac(EV['f1'], f1, f1_ps, relu=True)

                f2_ps = natps.tile([128, 2 * H], F32, tag="natps")
                for lt in range(2):
                    nc.tensor.matmul(f2_ps[:, lt * H:(lt + 1) * H],
                                     f1[:, lt * 128:(lt + 1) * 128], w_f2,
                                     start=True, stop=True)
                for lt in range(2):
                    sl = slice(lt * H, (lt + 1) * H)
                    nc.vector.tensor_tensor_reduce(
                        out=x2in[:, sl], in0=f2_ps[:, sl], in1=x1[:, sl],
                        scale=1.0, scalar=0.0, op0=ALU.add, op1=ALU.add,
                        accum_out=s2_g[:, 2 * kk + lt:2 * kk + lt + 1])
                for lt in range(2):
                    sl = slice(lt * H, (lt + 1) * H)
                    scr = sqp.tile([128, H], BF16, tag="scr")
                    nc.vector.tensor_tensor_reduce(
                        out=scr, in0=x2in[:, sl], in1=x2in[:, sl],
                        scale=1.0, scalar=0.0, op0=ALU.mult, op1=ALU.add,
                        accum_out=q2_g[:, 2 * kk + lt:2 * kk + lt + 1])

            def phase_b3(x2in, mean2, rstd2m, kk, pool_g):
                """LN2 apply (mask folded into rstd2m) + pooling matmuls."""
                x2 = x12p.tile([128, 2 * H], BF16, tag="x2")
                for lt in range(2):
                    sl = slice(lt * H, (lt + 1) * H)
                    nc.vector.tensor_scalar(
                        out=x2[:, sl], in0=x2in[:, sl],
                        scalar1=mean2[:, 2 * kk + lt:2 * kk + lt + 1],
                        scalar2=rstd2m[:, 2 * kk + lt:2 * kk + lt + 1],
                        op0=ALU.subtract, op1=ALU.mult)
                for lt in range(2):
                    nc.tensor.matmul(pool_g[:, kk:kk + 1],
                                     x2[:, lt * H:(lt + 1) * H], ones_b,
                                     start=(lt == 0), stop=(lt == 1))

            # persistent per-group-slot tiles (unique tags: all GRP alive)
            x1in_t = [persist.tile([128, 2 * H], F32, tag=f"x1in{i}",
                                   name=f"x1in_{i}") for i in range(GRP)]
            x2in_t = [persist.tile([128, 2 * H], BF16, tag=f"x2in{i}",
                                   name=f"x2in_{i}") for i in range(GRP)]

            # ---- per-group emission ----
            for g in range(NGRP):
                xs = xpool.tile([DSEQ, GRP * L], BF16, tag="X")
                nc.sync.dma_start(out=xs, in_=x_in[g, :, :])

                s1_g = lnp.tile([128, 2 * GRP], F32, tag="s1g")
                q1_g = lnp.tile([128, 2 * GRP], F32, tag="q1g")
                s2_g = lnp.tile([128, 2 * GRP], F32, tag="s2g")
                q2_g = lnp.tile([128, 2 * GRP], F32, tag="q2g")

                # A + B1 in micro-batches (den recip batched per MB units)
                for mb in range(GRP // MB):
                    kks = list(range(mb * MB, (mb + 1) * MB))
                    sts = [phase_a(xs, kk, g * GRP + kk) for kk in kks]
                    den_g = colps.tile([128, GRP], F32, tag="colg")
                    for i in range(MB):
                        den_mm(sts[i], den_g, i)
                    rec = small.tile([128, 2 * MB], F32, tag="rec")
                    nc.vector.reciprocal(rec, den_g[:, :2 * MB])
                    for i, kk in enumerate(kks):
                        phase_b1(sts[i], rec, i, s1_g, q1_g,
                                 x1in_t[kk], kk)

                mean1, rstd1 = ln_stats(s1_g, q1_g, 2 * GRP)
                for kk in range(GRP):
                    phase_b2(x1in_t[kk], mean1, rstd1, kk,
                             s2_g, q2_g, x2in_t[kk])

                mcols = m01_all[:, 2 * g * GRP:2 * (g + 1) * GRP]
                mean2, rstd2m = ln_stats(s2_g, q2_g, 2 * GRP, mask_cols=mcols)
                pool_g = colps.tile([128, GRP], F32, tag="colg")
                for kk in range(GRP):
                    phase_b3(x2in_t[kk], mean2, rstd2m, kk, pool_g)
                nc.vector.tensor_copy(pooled[:, g * GRP:(g + 1) * GRP], pool_g)

            # ---- per-core tail: unit_fc, building-sum, fusion MLP ----
            u16_ps = natps.tile([128, 2 * H], F32, tag="natps")
            nc.tensor.matmul(u16_ps[:UNITD, :NU], w_u, pooled,
                             start=True, stop=True)
            u16 = work.tile([UNITD, NU], F32, tag="u16")
            nc.scalar.activation(out=u16, in_=u16_ps[:UNITD, :NU],
                                 func=AF.Relu, bias=0.0, scale=1.0)

            u16t_ps = psB.tile([128, 2 * H], F32, tag="psB")
            nc.tensor.transpose(u16t_ps[:NU, :UNITD], u16,
                                ident[:UNITD, :UNITD])
            u16t = work.tile([NU, UNITD], BF16, tag="u16t")
            nc.vector.tensor_copy(u16t, u16t_ps[:NU, :UNITD])

            seq_ps = natps.tile([128, 2 * H], F32, tag="natps")
            nc.tensor.matmul(seq_ps[:UNITD, :BPC], u16t, s_sb,
                             start=True, stop=True)

            fused = work.tile([UNITD + AGGD + TODD, BPC], BF16, tag="fused")
            nc.vector.tensor_copy(fused[:UNITD, :], seq_ps[:UNITD, :BPC])
            nc.gpsimd.dma_start(out=fused[UNITD:, :], in_=tail_in[:, :])

            h1_ps = psB.tile([128, 2 * H], F32, tag="psB")
            nc.tensor.matmul(h1_ps[:H, :BPC], w_c1, fused,
                             start=True, stop=True)
            h1 = work.tile([H, BPC], BF16, tag="h1")
            nc.scalar.activation(out=h1, in_=h1_ps[:H, :BPC], func=AF.Relu,
                                 bias=0.0, scale=1.0)

            o_ps = natps.tile([128, 2 * H], F32, tag="natps")
            nc.tensor.matmul(o_ps[:DOUT, :BPC], w_c2, h1,
                             start=True, stop=True)
            o_s = work.tile([DOUT, BPC], F32, tag="osb")
            nc.scalar.activation(out=o_s, in_=o_ps[:DOUT, :BPC], func=AF.Relu,
                                 bias=0.0, scale=1.0)
            nc.sync.dma_start(out=out_t[:, :], in_=o_s)

    return nc


def _prep_weights(inputs):
    ipw = np.asarray(inputs["in_proj_w"])
    wts = {
        "w_inT": np.asarray(inputs["W_in"]).T,       # [5,128]
        "w_gT": (ipw[0:H] @ ipw[H:2 * H].T),          # Wq^T Wk composed [128,128]
        "w_vT": ipw[2 * H:3 * H].T,
        "w_oT": np.asarray(inputs["out_proj_w"]).T,
        "w_f1T": np.asarray(inputs["W_ff1"]).T,
        "w_f2T": np.asarray(inputs["W_ff2"]).T,
        "w_uT": np.asarray(inputs["W_unit"]).T,       # [128,16]
        "w_c1T": np.asarray(inputs["W_fc1"]).T,       # [26,128]
        "w_c2T": np.asarray(inputs["W_fc2"]).T,       # [128,128]
    }
    wts = {k: np.ascontiguousarray(v.astype(NPBF)) for k, v in wts.items()}
    # the kernel folds no biases / LN affines: assert they are trivial
    for nm in ("b_in", "in_proj_b", "out_proj_b", "b_ff1", "b_ff2",
               "ln1_b", "ln2_b", "b_unit", "b_fc1", "b_fc2"):
        assert np.max(np.abs(np.asarray(inputs[nm]))) == 0.0, f"{nm} nonzero"
    for nm in ("ln1_w", "ln2_w"):
        assert np.allclose(np.asarray(inputs[nm]), 1.0), f"{nm} nontrivial"
    return wts


def make_in_maps(inputs):
    x_seq = np.asarray(inputs["x_seq"], dtype=np.float32)       # [B,U,L,5]
    lengths = np.asarray(inputs["lengths"])                      # [B,U] int
    x_agg = np.asarray(inputs["x_agg_quant"], dtype=np.float32)  # [B,7]
    tod_emb = np.asarray(inputs["tod_emb"], dtype=np.float32)    # [5,3]
    tod_idx = np.asarray(inputs["tod_idx"])                      # [B] int

    in_maps = []
    for c in range(NCORES):
        bs = slice(c * BPC, (c + 1) * BPC)
        xc = x_seq[bs].reshape(NU, L, DSEQ).transpose(0, 2, 1)   # [128,5,256]
        xg = np.ascontiguousarray(
            xc.reshape(NGRP, GRP, DSEQ, L).transpose(0, 2, 1, 3)
            .reshape(NGRP, DSEQ, GRP * L)).astype(NPBF)
        lens = lengths[bs].reshape(NU).astype(np.float32)
        iota = np.arange(L, dtype=np.float32).reshape(2, 128).T  # [128p, 2 tiles]
        # resident mask tile [128p, NU*2]: col 2u+t = (p + 128t) < len[u]
        m01 = (iota[:, None, :] < lens[None, :, None]).astype(np.float32)
        m01 = np.ascontiguousarray(m01.reshape(128, NU * 2))
        eb = (1.0 - m01) * NEGB                                  # 0 valid / -30
        S = np.zeros((NU, BPC), np.float32)
        S[np.arange(NU), np.arange(NU) // U] = 1.0
        tail = np.concatenate(
            [x_agg[bs].T, tod_emb[tod_idx[bs]].T], axis=0)
        in_maps.append({"xg": xg, "m01": m01,
                        "eb": np.ascontiguousarray(eb),
                        "S": S.astype(NPBF),
                        "tail": np.ascontiguousarray(tail).astype(NPBF)})
    return in_maps


def kernel(_trace=False, **inputs):
    wts = _prep_weights(inputs)
    nc = build_nc(wts)
    if not nc.is_finalized():
        nc.finalize()
    in_maps = make_in_maps(inputs)
    res = run_bass_kernel_spmd(nc, in_maps, core_ids=list(range(NCORES)),
                               trace=_trace)
    out = np.zeros((B, DOUT), np.float32)
    for c in range(NCORES):
        out[c * BPC:(c + 1) * BPC, :] = res.results[c]["outT"].T
    if _trace:
        kernel._last_results = res
    return out


# revision 13
# speedup vs baseline: 1.5885x; 1.2625x over previous
"""Trainium2 Bass kernel for nn_DeliveryEventEncoder.

Pure data parallel across 8 NeuronCores (4 buildings = 128 units per core).
Activations kept feature-major [feat(128 part), seq(free)]; matmul inputs
bf16, fp32 PSUM accumulation.

Cost-model-driven design (TimelineSim):
 - Per-op fixed overheads dominate (ACT ~185ns, DVE ~60/125ns), so
   evacuations process unit PAIRS ([*, 512] tiles) and LayerNorm stats are
   group-batched: mean via free accum_out on the residual add, sumsq via
   DVE tensor_tensor_reduce, variance/sqrt/recip on [128, 2*GRP] tiles.
 - All sequencers are in-order and head-of-line block on semaphore waits,
   so emission is STAGE-MAJOR over micro-batches of 4 units: every
   consumer instruction sits several producers downstream, giving each
   engine slack.
 - Ragged key mask folds into the softmax exp bias (0/-30 per key row);
   query mask folds into LN2's rstd (zeroed rows vanish from sum-pool).
 - PSUM is bank-granular: psA x3 + psB x2 + psT x1 + natps x2 = 8 banks.
   den/pool column tiles share the natps tag.
"""

import os
import numpy as np
import ml_dtypes

import concourse.bass as bass
import concourse.bacc as bacc_mod
import concourse.mybir as mybir
import concourse.tile as tile
from concourse.bass_utils import run_bass_kernel_spmd
from concourse.masks import make_identity

F32 = mybir.dt.float32
BF16 = mybir.dt.bfloat16
AF = mybir.ActivationFunctionType
ALU = mybir.AluOpType
NPBF = ml_dtypes.bfloat16

B, U, L, DSEQ, H, DOUT = 32, 32, 256, 5, 128, 128
TODV, TODD, AGGD, UNITD = 5, 3, 7, 16
NCORES = 8
BPC = B // NCORES          # buildings per core
NU = BPC * U               # units per core (128)
GRP = int(os.environ.get('KGRP', '32'))  # units per phase block
NGRP = NU // GRP
MB = 4                     # units per micro-batch (2 pairs)
CSCALE = 1.0 / np.sqrt(H)
EPS = 1e-5
NEGB = -30.0               # exp bias for masked keys

# evacuation engine assignment (tunable)
EV = dict(embT='act', yT='pool', vs='pool', aoT='pool', x1T='dve',
          f1='act')
for kv in os.environ.get('KEV', '').split(','):
    if kv:
        k_, v_ = kv.split('=')
        EV[k_] = v_


def build_nc(wts):
    nc = bacc_mod.Bacc()

    x_in = nc.dram_tensor("xg", [NGRP, DSEQ, GRP * L], BF16, kind="ExternalInput")
    m01_in = nc.dram_tensor("m01", [128, NU * 2], F32, kind="ExternalInput")
    eb_in = nc.dram_tensor("eb", [128, NU * 2], F32, kind="ExternalInput")
    s_in = nc.dram_tensor("S", [NU, BPC], BF16, kind="ExternalInput")
    tail_in = nc.dram_tensor("tail", [AGGD + TODD, BPC], BF16, kind="ExternalInput")
    out_t = nc.dram_tensor("outT", [DOUT, BPC], F32, kind="ExternalOutput")

    dW = {k: nc.inline_tensor(v, name=k) for k, v in wts.items()}

    cfg = dict(xp=2, wk=3, sm=4, es=2, x12=3, xT=2, sq=2,
               ln=2, psA=3, psB=2, psT=1, nat=2)
    for kv in os.environ.get("KPOOLS", "").split(","):
        if kv:
            k_, v_ = kv.split("=")
            cfg[k_] = int(v_)

    def evac(engine, out, in_, relu=False):
        if engine == 'act':
            nc.scalar.activation(out=out, in_=in_,
                                 func=AF.Relu if relu else AF.Copy,
                                 bias=0.0, scale=1.0)
        elif engine == 'dve':
            if relu:
                nc.vector.tensor_scalar(out=out, in0=in_, scalar1=0.0,
                                        scalar2=None, op0=ALU.max)
            else:
                nc.vector.tensor_copy(out, in_)
        else:
            if relu:
                nc.gpsimd.tensor_scalar(out=out, in0=in_, scalar1=0.0,
                                        scalar2=None, op0=ALU.max)
            else:
                nc.gpsimd.tensor_copy(out, in_)

    with tile.TileContext(nc) as tc:
        with (
            tc.tile_pool(name="singles", bufs=1) as singles,
            tc.tile_pool(name="persist", bufs=1) as persist,
            tc.tile_pool(name="xpool", bufs=cfg["xp"]) as xpool,
            tc.tile_pool(name="work", bufs=cfg["wk"]) as work,
            tc.tile_pool(name="small", bufs=cfg["sm"]) as small,
            tc.tile_pool(name="espool", bufs=cfg["es"]) as espool,
            tc.tile_pool(name="x12p", bufs=cfg["x12"]) as x12p,
            tc.tile_pool(name="xTp", bufs=cfg["xT"]) as xTp,
            tc.tile_pool(name="sqp", bufs=cfg["sq"]) as sqp,
            tc.tile_pool(name="lnp", bufs=cfg["ln"]) as lnp,
            tc.tile_pool(name="psA", bufs=cfg["psA"], space="PSUM") as psA,
            tc.tile_pool(name="psB", bufs=cfg["psB"], space="PSUM") as psB,
            tc.tile_pool(name="psT", bufs=cfg["psT"], space="PSUM") as psT,
            tc.tile_pool(name="natps", bufs=cfg["nat"], space="PSUM") as natps,
        ):
            # ---- constants into SBUF ----
            def load_w(name, p, f):
                t = singles.tile([p, f], BF16, tag=name)
                nc.gpsimd.dma_start(out=t, in_=dW[name][:, :])
                return t

            w_in = load_w("w_inT", DSEQ, H)
            w_g = load_w("w_gT", H, H)
            w_v = load_w("w_vT", H, H)
            w_o = load_w("w_oT", H, H)
            w_f1 = load_w("w_f1T", H, H)
            w_f2 = load_w("w_f2T", H, H)
            w_u = load_w("w_uT", H, UNITD)
            w_c1 = load_w("w_c1T", UNITD + AGGD + TODD, H)
            w_c2 = load_w("w_c2T", H, DOUT)

            ident = singles.tile([128, 128], F32, tag="ident")
            make_identity(nc, ident)
            ident_b = singles.tile([128, 128], BF16, tag="identb")
            nc.vector.tensor_copy(ident_b, ident)
            ones_b = singles.tile([128, 1], BF16, tag="ones")
            nc.vector.memset(ones_b, 1.0)
            eps_col = singles.tile([128, 1], F32, tag="eps")
            nc.vector.memset(eps_col, EPS * H * H)

            s_sb = singles.tile([NU, BPC], BF16, tag="S")
            nc.gpsimd.dma_start(out=s_sb, in_=s_in[:, :])
            m01_all = singles.tile([128, NU * 2], F32, tag="m01")
            nc.gpsimd.dma_start(out=m01_all, in_=m01_in[:, :])
            eb_all = singles.tile([128, NU * 2], F32, tag="eb")
            nc.gpsimd.dma_start(out=eb_all, in_=eb_in[:, :])

            pooled = singles.tile([H, NU], BF16, tag="pooled")

            # persistent per-group-slot tiles (unique tags: all GRP alive)
            x1in_t = [persist.tile([128, 2 * H], F32, tag=f"x1in{i}",
                                   name=f"x1in_{i}") for i in range(GRP)]
            x2in_t = [persist.tile([128, 2 * H], BF16, tag=f"x2in{i}",
                                   name=f"x2in_{i}") for i in range(GRP)]

            def ln_stats(s_g, q_g, cols, mask_cols=None):
                """Batched LN stats: mean = s/H; rstd(+mask) =
                H / sqrt(H*q - s^2 + H^2 eps) [* mask]."""
                mean = lnp.tile([128, cols], F32, tag="mean")
                nc.vector.tensor_scalar(out=mean, in0=s_g, scalar1=1.0 / H,
                                        scalar2=None, op0=ALU.mult)
                sq = lnp.tile([128, cols], F32, tag="sq")
                nc.vector.tensor_tensor(out=sq, in0=s_g, in1=s_g, op=ALU.mult)
                var = lnp.tile([128, cols], F32, tag="var")
                nc.vector.scalar_tensor_tensor(
                    out=var, in0=q_g, scalar=float(H), in1=sq,
                    op0=ALU.mult, op1=ALU.subtract)
                sd = lnp.tile([128, cols], F32, tag="sd")
                nc.scalar.activation(out=sd, in_=var, func=AF.Sqrt,
                                     bias=eps_col, scale=1.0)
                rstd = lnp.tile([128, cols], F32, tag="rstd")
                nc.vector.reciprocal(rstd, sd)
                rstdm = lnp.tile([128, cols], F32, tag="rstdm")
                if mask_cols is not None:
                    nc.vector.scalar_tensor_tensor(
                        out=rstdm, in0=rstd, scalar=float(H), in1=mask_cols,
                        op0=ALU.mult, op1=ALU.mult)
                else:
                    nc.vector.tensor_scalar(out=rstdm, in0=rstd,
                                            scalar1=float(H), scalar2=None,
                                            op0=ALU.mult)
                return mean, rstdm

            # ---- per-group emission ----
            for g in range(NGRP):
                xs = xpool.tile([DSEQ, GRP * L], BF16, tag="X")
                nc.sync.dma_start(out=xs, in_=x_in[g, :, :])

                s1_g = lnp.tile([128, 2 * GRP], F32, tag="s1g")
                q1_g = lnp.tile([128, 2 * GRP], F32, tag="q1g")
                s2_g = lnp.tile([128, 2 * GRP], F32, tag="s2g")
                q2_g = lnp.tile([128, 2 * GRP], F32, tag="q2g")

                # ---------- A + B1, stage-major per micro-batch ----------
                for mb in range(GRP // MB):
                    u0 = mb * MB            # group-local first unit
                    pairs = [u0, u0 + 2]    # pair starts (2 units each)

                    embT, yT, vs = {}, {}, {}
                    for p in pairs:
                        emb_ps = psA.tile([128, 512], F32, tag="psA")
                        nc.tensor.matmul(emb_ps[:H, :],
                                         w_in, xs[:, p * L:(p + 2) * L],
                                         start=True, stop=True)
                        embT[p] = work.tile([H, 512], BF16, tag="embT", name=f"embT_{g}_{p}")
                        evac(EV['embT'], embT[p], emb_ps[:H, :])
                    for p in pairs:
                        y_ps = psA.tile([128, 512], F32, tag="psA")
                        nc.tensor.matmul(y_ps[:H, :], w_g, embT[p],
                                         start=True, stop=True)
                        yT[p] = work.tile([H, 512], BF16, tag="yT", name=f"yT_{g}_{p}")
                        evac(EV['yT'], yT[p], y_ps[:H, :])
                    for p in pairs:
                        v_ps = psA.tile([128, 512], F32, tag="psA")
                        for q in range(4):   # (iu, mt) quarters
                            nc.tensor.matmul(
                                v_ps[:, q * H:(q + 1) * H],
                                embT[p][:, q * 128:(q + 1) * 128], w_v,
                                start=True, stop=True)
                        vs[p] = work.tile([128, 512], BF16, tag="vs", name=f"vs_{g}_{p}")
                        evac(EV['vs'], vs[p], v_ps)

                    es = {}
                    for p in pairs:
                        for iu in range(2):
                            ug = p + iu     # group-local unit
                            u = g * GRP + ug
                            sc_ps = psA.tile([128, 512], F32, tag="psA")
                            for mt in range(2):
                                nc.tensor.matmul(
                                    sc_ps[:, mt * L:(mt + 1) * L],
                                    embT[p][:, (2 * iu + mt) * 128:
                                            (2 * iu + mt + 1) * 128],
                                    yT[p][:, iu * L:(iu + 1) * L],
                                    start=True, stop=True)
                            for mt in range(2):
                                e = espool.tile([128, L], BF16,
                                                tag=f"es{ug - u0}{mt}",
                                                name=f"es_{g}_{ug}_{mt}")
                                nc.scalar.activation(
                                    out=e, in_=sc_ps[:, mt * L:(mt + 1) * L],
                                    func=AF.Exp,
                                    bias=eb_all[:, 2 * u + mt:2 * u + mt + 1],
                                    scale=CSCALE)
                                es[(ug, mt)] = e

                    den_g = natps.tile([128, 512], F32, tag="natps")
                    for i in range(MB):
                        ug = u0 + i
                        for lt in range(2):
                            col = den_g[:, 2 * i + lt:2 * i + lt + 1]
                            for mt in range(2):
                                nc.tensor.matmul(
                                    col,
                                    es[(ug, mt)][:, lt * 128:(lt + 1) * 128],
                                    ones_b, start=(mt == 0), stop=(mt == 1))
                    rec = small.tile([128, 2 * MB], F32, tag="rec")
                    nc.vector.reciprocal(rec, den_g[:, :2 * MB])

                    aoT, en_t, pon_t = {}, {}, {}
                    for p in pairs:
                        ao_ps = psB.tile([H, 512], F32, tag="psB")
                        for iu in range(2):
                            for mt in range(2):
                                nc.tensor.matmul(
                                    ao_ps[:, iu * L:(iu + 1) * L],
                                    vs[p][:, (2 * iu + mt) * H:
                                          (2 * iu + mt + 1) * H],
                                    es[(p + iu, mt)],
                                    start=(mt == 0), stop=(mt == 1))
                        aoT[p] = work.tile([H, 512], BF16, tag="aoT", name=f"aoT_{g}_{p}")
                        evac(EV['aoT'], aoT[p], ao_ps)
                    for p in pairs:
                        en_ps = psA.tile([128, 512], F32, tag="psA")
                        for q in range(4):
                            nc.tensor.matmul(
                                en_ps[:, q * H:(q + 1) * H],
                                xs[:, p * L + q * 128:p * L + (q + 1) * 128],
                                w_in, start=True, stop=True)
                        en_t[p] = en_ps
                    for p in pairs:
                        pon_ps = natps.tile([128, 512], F32, tag="natps")
                        for q in range(4):
                            nc.tensor.matmul(
                                pon_ps[:, q * H:(q + 1) * H],
                                aoT[p][:, q * 128:(q + 1) * 128], w_o,
                                start=True, stop=True)
                        pon_t[p] = pon_ps
                    for p in pairs:
                        for iu in range(2):
                            ug = p + iu
                            x1in = x1in_t[ug]
                            for lt in range(2):
                                q = 2 * iu + lt
                                nc.vector.scalar_tensor_tensor(
                                    out=x1in[:, lt * H:(lt + 1) * H],
                                    in0=pon_t[p][:, q * H:(q + 1) * H],
                                    scalar=rec[:, 2 * (ug - u0) + lt:
                                               2 * (ug - u0) + lt + 1],
                                    in1=en_t[p][:, q * H:(q + 1) * H],
                                    op0=ALU.mult, op1=ALU.add,
                                    accum_out=s1_g[:, 2 * ug + lt:
                                                   2 * ug + lt + 1])
                    for p in pairs:
                        for iu in range(2):
                            ug = p + iu
                            x1in = x1in_t[ug]
                            for lt in range(2):
                                scr = sqp.tile([128, H], BF16, tag="scr")
                                nc.vector.tensor_tensor_reduce(
                                    out=scr, in0=x1in[:, lt * H:(lt + 1) * H],
                                    in1=x1in[:, lt * H:(lt + 1) * H],
                                    scale=1.0, scalar=0.0,
                                    op0=ALU.mult, op1=ALU.add,
                                    accum_out=q1_g[:, 2 * ug + lt:
                                                   2 * ug + lt + 1])

                mean1, rstd1 = ln_stats(s1_g, q1_g, 2 * GRP)

                # ---------- B2, stage-major per 2-pair block ----------
                x1_t, x1T_t, f1_t = {}, {}, {}
                for blk in range(GRP // 4):     # 4 units = 2 pairs per blk
                    b0 = blk * 4
                    for p in (b0, b0 + 2):
                        x1 = x12p.tile([128, 512], BF16, tag="x1",
                                       name=f"x1_{g}_{p}")
                        for iu in range(2):
                            ug = p + iu
                            for lt in range(2):
                                q = 2 * iu + lt
                                nc.vector.tensor_scalar(
                                    out=x1[:, q * H:(q + 1) * H],
                                    in0=x1in_t[ug][:, lt * H:(lt + 1) * H],
                                    scalar1=mean1[:, 2 * ug + lt:
                                                  2 * ug + lt + 1],
                                    scalar2=rstd1[:, 2 * ug + lt:
                                                  2 * ug + lt + 1],
                                    op0=ALU.subtract, op1=ALU.mult)
                        x1_t[p] = x1
                    x1t_ps = psT.tile([H, 1024], BF16, tag="psT")
                    for j, p in enumerate((b0, b0 + 2)):
                        for q in range(4):
                            nc.tensor.transpose(
                                x1t_ps[:, j * 512 + q * 128:
                                       j * 512 + (q + 1) * 128],
                                x1_t[p][:, q * H:(q + 1) * H], ident_b)
                    x1T = xTp.tile([H, 1024], BF16, tag="x1T")
                    evac(EV['x1T'], x1T, x1t_ps)
                    for j, p in enumerate((b0, b0 + 2)):
                        f1_ps = psB.tile([H, 512], F32, tag="psB")
                        nc.tensor.matmul(f1_ps, w_f1,
                                         x1T[:, j * 512:(j + 1) * 512],
                                         start=True, stop=True)
                        f1 = work.tile([H, 512], BF16, tag="f1")
                        evac(EV['f1'], f1, f1_ps, relu=True)
                        f1_t[p] = f1
                    for p in (b0, b0 + 2):
                        f2_ps = natps.tile([128, 512], F32, tag="natps")
                        for q in range(4):
                            nc.tensor.matmul(
                                f2_ps[:, q * H:(q + 1) * H],
                                f1_t[p][:, q * 128:(q + 1) * 128], w_f2,
                                start=True, stop=True)
                        for iu in range(2):
                            ug = p + iu
                            for lt in range(2):
                                q = 2 * iu + lt
                                nc.vector.tensor_tensor_reduce(
                                    out=x2in_t[ug][:, lt * H:(lt + 1) * H],
                                    in0=f2_ps[:, q * H:(q + 1) * H],
                                    in1=x1_t[p][:, q * H:(q + 1) * H],
                                    scale=1.0, scalar=0.0,
                                    op0=ALU.add, op1=ALU.add,
                                    accum_out=s2_g[:, 2 * ug + lt:
                                                   2 * ug + lt + 1])
                    for p in (b0, b0 + 2):
                        for iu in range(2):
                            ug = p + iu
                            for lt in range(2):
                                scr = sqp.tile([128, H], BF16, tag="scr")
                                nc.vector.tensor_tensor_reduce(
                                    out=scr,
                                    in0=x2in_t[ug][:, lt * H:(lt + 1) * H],
                                    in1=x2in_t[ug][:, lt * H:(lt + 1) * H],
                                    scale=1.0, scalar=0.0,
                                    op0=ALU.mult, op1=ALU.add,
                                    accum_out=q2_g[:, 2 * ug + lt:
                                                   2 * ug + lt + 1])

                mcols = m01_all[:, 2 * g * GRP:2 * (g + 1) * GRP]
                mean2, rstd2m = ln_stats(s2_g, q2_g, 2 * GRP, mask_cols=mcols)

                # ---------- B3 ----------
                pool_g = natps.tile([128, 512], F32, tag="natps")
                for p in range(0, GRP, 2):
                    x2 = x12p.tile([128, 512], BF16, tag="x2",
                                   name=f"x2_{g}_{p}")
                    for iu in range(2):
                        ug = p + iu
                        for lt in range(2):
                            q = 2 * iu + lt
                            nc.vector.tensor_scalar(
                                out=x2[:, q * H:(q + 1) * H],
                                in0=x2in_t[ug][:, lt * H:(lt + 1) * H],
                                scalar1=mean2[:, 2 * ug + lt:2 * ug + lt + 1],
                                scalar2=rstd2m[:, 2 * ug + lt:2 * ug + lt + 1],
                                op0=ALU.subtract, op1=ALU.mult)
                    for iu in range(2):
                        ug = p + iu
                        for lt in range(2):
                            nc.tensor.matmul(
                                pool_g[:, ug:ug + 1],
                                x2[:, (2 * iu + lt) * H:(2 * iu + lt + 1) * H],
                                ones_b, start=(lt == 0), stop=(lt == 1))
                nc.vector.tensor_copy(pooled[:, g * GRP:(g + 1) * GRP],
                                      pool_g[:H, :GRP])

            # ---- per-core tail: unit_fc, building-sum, fusion MLP ----
            u16_ps = natps.tile([128, 512], F32, tag="natps")
            nc.tensor.matmul(u16_ps[:UNITD, :NU], w_u, pooled,
                             start=True, stop=True)
            u16 = work.tile([UNITD, NU], F32, tag="u16")
            nc.scalar.activation(out=u16, in_=u16_ps[:UNITD, :NU],
                                 func=AF.Relu, bias=0.0, scale=1.0)

            u16t_ps = psB.tile([H, 512], F32, tag="psB")
            nc.tensor.transpose(u16t_ps[:NU, :UNITD], u16,
                                ident[:UNITD, :UNITD])
            u16t = work.tile([NU, UNITD], BF16, tag="u16t")
            nc.vector.tensor_copy(u16t, u16t_ps[:NU, :UNITD])

            seq_ps = natps.tile([128, 512], F32, tag="natps")
            nc.tensor.matmul(seq_ps[:UNITD, :BPC], u16t, s_sb,
                             start=True, stop=True)

            fused = work.tile([UNITD + AGGD + TODD, BPC], BF16, tag="fused")
            nc.vector.tensor_copy(fused[:UNITD, :], seq_ps[:UNITD, :BPC])
            nc.gpsimd.dma_start(out=fused[UNITD:, :], in_=tail_in[:, :])

            h1_ps = psB.tile([H, 512], F32, tag="psB")
            nc.tensor.matmul(h1_ps[:H, :BPC], w_c1, fused,
                             start=True, stop=True)
            h1 = work.tile([H, BPC], BF16, tag="h1")
            nc.scalar.activation(out=h1, in_=h1_ps[:H, :BPC], func=AF.Relu,
                                 bias=0.0, scale=1.0)

            o_ps = natps.tile([128, 512], F32, tag="natps")
            nc.tensor.matmul(o_ps[:DOUT, :BPC], w_c2, h1,
                             start=True, stop=True)
            o_s = work.tile([DOUT, BPC], F32, tag="osb")
            nc.scalar.activation(out=o_s, in_=o_ps[:DOUT, :BPC], func=AF.Relu,
                                 bias=0.0, scale=1.0)
            nc.sync.dma_start(out=out_t[:, :], in_=o_s)

    return nc


def _prep_weights(inputs):
    ipw = np.asarray(inputs["in_proj_w"])
    wts = {
        "w_inT": np.asarray(inputs["W_in"]).T,       # [5,128]
        "w_gT": (ipw[0:H] @ ipw[H:2 * H].T),          # Wq^T Wk composed [128,128]
        "w_vT": ipw[2 * H:3 * H].T,
        "w_oT": np.asarray(inputs["out_proj_w"]).T,
        "w_f1T": np.asarray(inputs["W_ff1"]).T,
        "w_f2T": np.asarray(inputs["W_ff2"]).T,
        "w_uT": np.asarray(inputs["W_unit"]).T,       # [128,16]
        "w_c1T": np.asarray(inputs["W_fc1"]).T,       # [26,128]
        "w_c2T": np.asarray(inputs["W_fc2"]).T,       # [128,128]
    }
    wts = {k: np.ascontiguousarray(v.astype(NPBF)) for k, v in wts.items()}
    # the kernel folds no biases / LN affines: assert they are trivial
    for nm in ("b_in", "in_proj_b", "out_proj_b", "b_ff1", "b_ff2",
               "ln1_b", "ln2_b", "b_unit", "b_fc1", "b_fc2"):
        assert np.max(np.abs(np.asarray(inputs[nm]))) == 0.0, f"{nm} nonzero"
    for nm in ("ln1_w", "ln2_w"):
        assert np.allclose(np.asarray(inputs[nm]), 1.0), f"{nm} nontrivial"
    return wts


def make_in_maps(inputs):
    x_seq = np.asarray(inputs["x_seq"], dtype=np.float32)       # [B,U,L,5]
    lengths = np.asarray(inputs["lengths"])                      # [B,U] int
    x_agg = np.asarray(inputs["x_agg_quant"], dtype=np.float32)  # [B,7]
    tod_emb = np.asarray(inputs["tod_emb"], dtype=np.float32)    # [5,3]
    tod_idx = np.asarray(inputs["tod_idx"])                      # [B] int

    in_maps = []
    for c in range(NCORES):
        bs = slice(c * BPC, (c + 1) * BPC)
        xc = x_seq[bs].reshape(NU, L, DSEQ).transpose(0, 2, 1)   # [128,5,256]
        xg = np.ascontiguousarray(
            xc.reshape(NGRP, GRP, DSEQ, L).transpose(0, 2, 1, 3)
            .reshape(NGRP, DSEQ, GRP * L)).astype(NPBF)
        lens = lengths[bs].reshape(NU).astype(np.float32)
        iota = np.arange(L, dtype=np.float32).reshape(2, 128).T  # [128p, 2 tiles]
        # resident mask tile [128p, NU*2]: col 2u+t = (p + 128t) < len[u]
        m01 = (iota[:, None, :] < lens[None, :, None]).astype(np.float32)
        m01 = np.ascontiguousarray(m01.reshape(128, NU * 2))
        eb = (1.0 - m01) * NEGB                                  # 0 valid / -30
        S = np.zeros((NU, BPC), np.float32)
        S[np.arange(NU), np.arange(NU) // U] = 1.0
        tail = np.concatenate(
            [x_agg[bs].T, tod_emb[tod_idx[bs]].T], axis=0)
        in_maps.append({"xg": xg, "m01": m01,
                        "eb": np.ascontiguousarray(eb),
                        "S": S.astype(NPBF),
                        "tail": np.ascontiguousarray(tail).astype(NPBF)})
    return in_maps


def kernel(_trace=False, **inputs):
    wts = _prep_weights(inputs)
    nc = build_nc(wts)
    if not nc.is_finalized():
        nc.finalize()
    in_maps = make_in_maps(inputs)
    res = run_bass_kernel_spmd(nc, in_maps, core_ids=list(range(NCORES)),
                               trace=_trace)
    out = np.zeros((B, DOUT), np.float32)
    for c in range(NCORES):
        out[c * BPC:(c + 1) * BPC, :] = res.results[c]["outT"].T
    if _trace:
        kernel._last_results = res
    return out


# revision 17
# speedup vs baseline: 2.0333x; 1.2800x over previous
"""Trainium2 Bass kernel for nn_DeliveryEventEncoder.

Pure data parallel across 8 NeuronCores (4 buildings = 128 units per core).
Activations feature-major [feat(128 part), seq(free)]; bf16 matmul inputs,
fp32 PSUM accumulation.

Cost-model-driven design (TimelineSim):
 - Ragged clipping: units are sorted by length per core (host-side
   permutation, absorbed into the S pooling matrix and mask columns), the
   SPMD schedule is specialized to the slot-wise max length across cores
   (rounded to 8). Column-proportional work drops ~0.57x, per-chunk op
   count ~0.78x.
 - Per-op fixed overheads dominate (ACT ~185ns, DVE ~60/125ns), so
   evacuations process unit PAIRS and LayerNorm stats are group-batched:
   mean via free accum_out on the residual add, sumsq via DVE
   tensor_tensor_reduce, variance/sqrt/recip on [128, 2*GRP] tiles.
 - All sequencers are in-order and head-of-line block on semaphore waits,
   so emission is STAGE-MAJOR over micro-batches of 4 units.
 - Ragged key mask folds into the softmax exp bias (0/-30 per key row);
   query mask folds into LN2's rstd (zeroed rows vanish from sum-pool).
 - PSUM is bank-granular: psA x3 + psB x2 + psT x1 + natps x2 = 8 banks.
   den/pool column tiles share the natps tag.
"""

import os
import numpy as np
import ml_dtypes

import concourse.bass as bass
import concourse.bacc as bacc_mod
import concourse.mybir as mybir
import concourse.tile as tile
from concourse.bass_utils import run_bass_kernel_spmd
from concourse.masks import make_identity

F32 = mybir.dt.float32
BF16 = mybir.dt.bfloat16
AF = mybir.ActivationFunctionType
ALU = mybir.AluOpType
NPBF = ml_dtypes.bfloat16

B, U, L, DSEQ, H, DOUT = 32, 32, 256, 5, 128, 128
TODV, TODD, AGGD, UNITD = 5, 3, 7, 16
NCORES = 8
BPC = B // NCORES          # buildings per core
NU = BPC * U               # units per core (128)
GRP = 32                   # units per phase block
NGRP = NU // GRP
MB = 4                     # units per micro-batch (2 pairs)
CSCALE = 1.0 / np.sqrt(H)
EPS = 1e-5
NEGB = -30.0               # exp bias for masked keys

# Slot-max schedule lengths (units sorted desc per core, max across cores,
# rounded up to 8). Default matches reference.setup_inputs(); kernel()
# recomputes from the actual lengths at run time.
DEFAULT_SLENS = [
    256, 256, 256, 256, 256, 256, 256, 256, 256, 248, 248, 248, 248, 240,
    240, 240, 240, 240, 232, 232, 224, 224, 224, 224, 216, 216, 216, 216,
    216, 208, 208, 208, 208, 208, 208, 200, 200, 200, 200, 192, 192, 184,
    184, 176, 176, 176, 176, 176, 168, 168, 168, 168, 168, 168, 168, 168,
    160, 160, 160, 152, 152, 152, 144, 144, 144, 144, 136, 136, 136, 136,
    136, 128, 128, 128, 128, 128, 120, 120, 120, 120, 120, 120, 112, 112,
    104, 104, 104, 104, 104, 96, 96, 96, 96, 88, 88, 88, 80, 80, 80, 80,
    80, 80, 80, 72, 72, 72, 72, 72, 64, 64, 56, 56, 56, 56, 56, 48, 40,
    32, 32, 32, 32, 24, 24, 24, 16, 16, 16, 16]


def _slens_from_lengths(lengths):
    per_core = [np.sort(np.asarray(lengths)[c * BPC:(c + 1) * BPC]
                        .reshape(NU))[::-1] for c in range(NCORES)]
    slotmax = np.stack(per_core).max(axis=0)
    return np.minimum(L, ((slotmax + 7) // 8) * 8).astype(int).tolist()


def _sched(slens):
    """Per-slot schedule: ncols, chunk count, chunk widths, packed offsets
    (group-relative)."""
    ncols = [int(c) for c in slens]
    nck = [2 if c > 128 else 1 for c in ncols]
    ck = [[min(128, c), max(0, c - 128)] for c in ncols]
    go = []
    for g in range(NGRP):
        off, offs = 0, []
        for i in range(GRP):
            offs.append(off)
            off += ncols[g * GRP + i]
        go.append(offs)
    return ncols, nck, ck, go


# evacuation engine assignment (tunable)
EV = dict(embT='act', yT='pool', vs='pool', aoT='pool', x1T='dve',
          f1='act')
for kv in os.environ.get('KEV', '').split(','):
    if kv:
        k_, v_ = kv.split('=')
        EV[k_] = v_


def build_nc(wts, slens=None):
    if slens is None:
        slens = DEFAULT_SLENS
    ncols, nck, ck, go = _sched(slens)

    nc = bacc_mod.Bacc()

    x_in = nc.dram_tensor("xg", [NGRP, DSEQ, GRP * L], BF16, kind="ExternalInput")
    m01_in = nc.dram_tensor("m01", [128, NU * 2], F32, kind="ExternalInput")
    eb_in = nc.dram_tensor("eb", [128, NU * 2], F32, kind="ExternalInput")
    s_in = nc.dram_tensor("S", [NU, BPC], BF16, kind="ExternalInput")
    tail_in = nc.dram_tensor("tail", [AGGD + TODD, BPC], BF16, kind="ExternalInput")
    out_t = nc.dram_tensor("outT", [DOUT, BPC], F32, kind="ExternalOutput")

    dW = {k: nc.inline_tensor(v, name=k) for k, v in wts.items()}

    cfg = dict(xp=2, wk=3, sm=4, es=2, x12=3, xT=2, sq=2,
               ln=2, psA=3, psB=2, psT=1, nat=2)
    for kv in os.environ.get("KPOOLS", "").split(","):
        if kv:
            k_, v_ = kv.split("=")
            cfg[k_] = int(v_)

    def evac(engine, out, in_, relu=False):
        if engine == 'act':
            nc.scalar.activation(out=out, in_=in_,
                                 func=AF.Relu if relu else AF.Copy,
                                 bias=0.0, scale=1.0)
        elif engine == 'dve':
            if relu:
                nc.vector.tensor_scalar(out=out, in0=in_, scalar1=0.0,
                                        scalar2=None, op0=ALU.max)
            else:
                nc.vector.tensor_copy(out, in_)
        else:
            if relu:
                nc.gpsimd.tensor_scalar(out=out, in0=in_, scalar1=0.0,
                                        scalar2=None, op0=ALU.max)
            else:
                nc.gpsimd.tensor_copy(out, in_)

    with tile.TileContext(nc) as tc:
        with (
            tc.tile_pool(name="singles", bufs=1) as singles,
            tc.tile_pool(name="persist", bufs=1) as persist,
            tc.tile_pool(name="xpool", bufs=cfg["xp"]) as xpool,
            tc.tile_pool(name="work", bufs=cfg["wk"]) as work,
            tc.tile_pool(name="small", bufs=cfg["sm"]) as small,
            tc.tile_pool(name="espool", bufs=cfg["es"]) as espool,
            tc.tile_pool(name="x12p", bufs=cfg["x12"]) as x12p,
            tc.tile_pool(name="xTp", bufs=cfg["xT"]) as xTp,
            tc.tile_pool(name="sqp", bufs=cfg["sq"]) as sqp,
            tc.tile_pool(name="lnp", bufs=cfg["ln"]) as lnp,
            tc.tile_pool(name="statp", bufs=1) as statp,
            tc.tile_pool(name="psA", bufs=cfg["psA"], space="PSUM") as psA,
            tc.tile_pool(name="psB", bufs=cfg["psB"], space="PSUM") as psB,
            tc.tile_pool(name="psT", bufs=cfg["psT"], space="PSUM") as psT,
            tc.tile_pool(name="natps", bufs=cfg["nat"], space="PSUM") as natps,
        ):
            # ---- constants into SBUF ----
            def load_w(name, p, f):
                t = singles.tile([p, f], BF16, tag=name)
                nc.gpsimd.dma_start(out=t, in_=dW[name][:, :])
                return t

            w_in = load_w("w_inT", DSEQ, H)
            w_g = load_w("w_gT", H, H)
            w_v = load_w("w_vT", H, H)
            w_o = load_w("w_oT", H, H)
            w_f1 = load_w("w_f1T", H, H)
            w_f2 = load_w("w_f2T", H, H)
            w_u = load_w("w_uT", H, UNITD)
            w_c1 = load_w("w_c1T", UNITD + AGGD + TODD, H)
            w_c2 = load_w("w_c2T", H, DOUT)

            ident = singles.tile([128, 128], F32, tag="ident")
            make_identity(nc, ident)
            ident_b = singles.tile([128, 128], BF16, tag="identb")
            nc.vector.tensor_copy(ident_b, ident)
            ones_b = singles.tile([128, 1], BF16, tag="ones")
            nc.vector.memset(ones_b, 1.0)
            eps_col = singles.tile([128, 1], F32, tag="eps")
            nc.vector.memset(eps_col, EPS * H * H)

            s_sb = singles.tile([NU, BPC], BF16, tag="S")
            nc.gpsimd.dma_start(out=s_sb, in_=s_in[:, :])
            m01_all = singles.tile([128, NU * 2], F32, tag="m01")
            nc.gpsimd.dma_start(out=m01_all, in_=m01_in[:, :])
            eb_all = singles.tile([128, NU * 2], F32, tag="eb")
            nc.gpsimd.dma_start(out=eb_all, in_=eb_in[:, :])

            pooled = singles.tile([H, NU], BF16, tag="pooled")

            # persistent per-group-slot tiles (unique tags: all GRP alive)
            x1in_t = [persist.tile([128, 2 * H], F32, tag=f"x1in{i}",
                                   name=f"x1in_{i}") for i in range(GRP)]
            x2in_t = [persist.tile([128, 2 * H], BF16, tag=f"x2in{i}",
                                   name=f"x2in_{i}") for i in range(GRP)]

            # group stat accumulators: bufs=1 + memset once so rows beyond a
            # slot's chunk width hold stale-but-consistent (s, q) pairs
            s1_g = statp.tile([128, 2 * GRP], F32, tag="s1g")
            q1_g = statp.tile([128, 2 * GRP], F32, tag="q1g")
            s2_g = statp.tile([128, 2 * GRP], F32, tag="s2g")
            q2_g = statp.tile([128, 2 * GRP], F32, tag="q2g")
            for t in (s1_g, q1_g, s2_g, q2_g):
                nc.vector.memset(t, 0.0)

            def ln_stats(s_g, q_g, cols, mask_cols=None):
                """Batched LN stats: mean = s/H; rstd(+mask) =
                H / sqrt(H*q - s^2 + H^2 eps) [* mask]."""
                mean = lnp.tile([128, cols], F32, tag="mean")
                nc.vector.tensor_scalar(out=mean, in0=s_g, scalar1=1.0 / H,
                                        scalar2=None, op0=ALU.mult)
                sq = lnp.tile([128, cols], F32, tag="sq")
                nc.vector.tensor_tensor(out=sq, in0=s_g, in1=s_g, op=ALU.mult)
                var = lnp.tile([128, cols], F32, tag="var")
                nc.vector.scalar_tensor_tensor(
                    out=var, in0=q_g, scalar=float(H), in1=sq,
                    op0=ALU.mult, op1=ALU.subtract)
                sd = lnp.tile([128, cols], F32, tag="sd")
                nc.scalar.activation(out=sd, in_=var, func=AF.Sqrt,
                                     bias=eps_col, scale=1.0)
                rstd = lnp.tile([128, cols], F32, tag="rstd")
                nc.vector.reciprocal(rstd, sd)
                rstdm = lnp.tile([128, cols], F32, tag="rstdm")
                if mask_cols is not None:
                    nc.vector.scalar_tensor_tensor(
                        out=rstdm, in0=rstd, scalar=float(H), in1=mask_cols,
                        op0=ALU.mult, op1=ALU.mult)
                else:
                    nc.vector.tensor_scalar(out=rstdm, in0=rstd,
                                            scalar1=float(H), scalar2=None,
                                            op0=ALU.mult)
                return mean, rstdm

            # ---- per-group emission ----
            for g in range(NGRP):
                def NC_(i):
                    return ncols[g * GRP + i]

                def NK_(i):
                    return nck[g * GRP + i]

                def CW_(i, t):
                    return ck[g * GRP + i][t]

                gcols = sum(NC_(i) for i in range(GRP))

                xs = xpool.tile([DSEQ, GRP * L], BF16, tag="X")
                nc.sync.dma_start(out=xs[:, :gcols], in_=x_in[g, :, :gcols])

                # ---------- A + B1, stage-major per micro-batch ----------
                for mb in range(GRP // MB):
                    u0 = mb * MB
                    pairs = [u0, u0 + 2]

                    def cpair(p):
                        return NC_(p) + NC_(p + 1)

                    def qi(p, iu, t):      # chunk quarter index in pair
                        return NK_(p) * iu + t

                    def aoff(p, iu):       # col offset of unit iu in pair
                        return NC_(p) * iu

                    embT, yT, vs = {}, {}, {}
                    for p in pairs:
                        emb_ps = psA.tile([128, 512], F32, tag="psA")
                        nc.tensor.matmul(
                            emb_ps[:H, :cpair(p)], w_in,
                            xs[:, go[g][p]:go[g][p] + cpair(p)],
                            start=True, stop=True)
                        embT[p] = work.tile([H, 512], BF16, tag="embT",
                                            name=f"embT_{g}_{p}")
                        evac(EV['embT'], embT[p][:, :cpair(p)],
                             emb_ps[:H, :cpair(p)])
                    for p in pairs:
                        y_ps = psA.tile([128, 512], F32, tag="psA")
                        nc.tensor.matmul(y_ps[:H, :cpair(p)], w_g,
                                         embT[p][:, :cpair(p)],
                                         start=True, stop=True)
                        yT[p] = work.tile([H, 512], BF16, tag="yT",
                                          name=f"yT_{g}_{p}")
                        evac(EV['yT'], yT[p][:, :cpair(p)],
                             y_ps[:H, :cpair(p)])
                    for p in pairs:
                        nq = NK_(p) + NK_(p + 1)
                        v_ps = psA.tile([128, 512], F32, tag="psA")
                        for iu in range(2):
                            for t in range(NK_(p + iu)):
                                w = CW_(p + iu, t)
                                q = qi(p, iu, t)
                                nc.tensor.matmul(
                                    v_ps[:w, q * H:(q + 1) * H],
                                    embT[p][:, aoff(p, iu) + t * 128:
                                            aoff(p, iu) + t * 128 + w],
                                    w_v, start=True, stop=True)
                        vs[p] = work.tile([128, 512], BF16, tag="vs",
                                          name=f"vs_{g}_{p}")
                        evac(EV['vs'], vs[p][:, :nq * H], v_ps[:, :nq * H])

                    es = {}
                    for p in pairs:
                        for iu in range(2):
                            ug = p + iu
                            u = g * GRP + ug
                            cn = NC_(ug)
                            sc_ps = psA.tile([128, 512], F32, tag="psA")
                            for mt in range(NK_(ug)):
                                w = CW_(ug, mt)
                                nc.tensor.matmul(
                                    sc_ps[:w, mt * L:mt * L + cn],
                                    embT[p][:, aoff(p, iu) + mt * 128:
                                            aoff(p, iu) + mt * 128 + w],
                                    yT[p][:, aoff(p, iu):aoff(p, iu) + cn],
                                    start=True, stop=True)
                            for mt in range(NK_(ug)):
                                w = CW_(ug, mt)
                                e = espool.tile([128, L], BF16,
                                                tag=f"es{ug - u0}{mt}",
                                                name=f"es_{g}_{ug}_{mt}")
                                nc.scalar.activation(
                                    out=e[:w, :cn],
                                    in_=sc_ps[:w, mt * L:mt * L + cn],
                                    func=AF.Exp,
                                    bias=eb_all[:w, 2 * u + mt:2 * u + mt + 1],
                                    scale=CSCALE)
                                es[(ug, mt)] = e

                    # den columns: lt=0 -> col i; lt=1 -> col MB + i
                    # (sorted slots => nck=2 is a prefix within the batch)
                    den_g = natps.tile([128, 512], F32, tag="natps")
                    n2 = sum(1 for i in range(MB) if NK_(u0 + i) == 2)
                    for i in range(MB):
                        ug = u0 + i
                        for lt in range(NK_(ug)):
                            lw = CW_(ug, lt)
                            col = i if lt == 0 else MB + i
                            for mt in range(NK_(ug)):
                                w = CW_(ug, mt)
                                nc.tensor.matmul(
                                    den_g[:lw, col:col + 1],
                                    es[(ug, mt)][:w, lt * 128:lt * 128 + lw],
                                    ones_b[:w], start=(mt == 0),
                                    stop=(mt == NK_(ug) - 1))
                    rec = small.tile([128, 2 * MB], F32, tag="rec")
                    nc.vector.reciprocal(rec[:, :MB + n2],
                                         den_g[:, :MB + n2])

                    aoT, en_t, pon_t = {}, {}, {}
                    for p in pairs:
                        ao_ps = psB.tile([H, 512], F32, tag="psB")
                        for iu in range(2):
                            ug = p + iu
                            cn = NC_(ug)
                            for mt in range(NK_(ug)):
                                w = CW_(ug, mt)
                                nc.tensor.matmul(
                                    ao_ps[:, aoff(p, iu):aoff(p, iu) + cn],
                                    vs[p][:w, qi(p, iu, mt) * H:
                                          (qi(p, iu, mt) + 1) * H],
                                    es[(ug, mt)][:w, :cn],
                                    start=(mt == 0), stop=(mt == NK_(ug) - 1))
                        aoT[p] = work.tile([H, 512], BF16, tag="aoT",
                                           name=f"aoT_{g}_{p}")
                        evac(EV['aoT'], aoT[p][:, :cpair(p)],
                             ao_ps[:, :cpair(p)])
                    for p in pairs:
                        en_ps = psA.tile([128, 512], F32, tag="psA")
                        for iu in range(2):
                            ug = p + iu
                            for lt in range(NK_(ug)):
                                w = CW_(ug, lt)
                                q = qi(p, iu, lt)
                                nc.tensor.matmul(
                                    en_ps[:w, q * H:(q + 1) * H],
                                    xs[:, go[g][p] + aoff(p, iu) + lt * 128:
                                       go[g][p] + aoff(p, iu) + lt * 128 + w],
                                    w_in, start=True, stop=True)
                        en_t[p] = en_ps
                    for p in pairs:
                        pon_ps = natps.tile([128, 512], F32, tag="natps")
                        for iu in range(2):
                            ug = p + iu
                            for lt in range(NK_(ug)):
                                w = CW_(ug, lt)
                                q = qi(p, iu, lt)
                                nc.tensor.matmul(
                                    pon_ps[:w, q * H:(q + 1) * H],
                                    aoT[p][:, aoff(p, iu) + lt * 128:
                                           aoff(p, iu) + lt * 128 + w],
                                    w_o, start=True, stop=True)
                        pon_t[p] = pon_ps
                    for p in pairs:
                        for iu in range(2):
                            ug = p + iu
                            i = ug - u0
                            x1in = x1in_t[ug]
                            for lt in range(NK_(ug)):
                                w = CW_(ug, lt)
                                q = qi(p, iu, lt)
                                rcol = i if lt == 0 else MB + i
                                nc.vector.scalar_tensor_tensor(
                                    out=x1in[:w, lt * H:(lt + 1) * H],
                                    in0=pon_t[p][:w, q * H:(q + 1) * H],
                                    scalar=rec[:w, rcol:rcol + 1],
                                    in1=en_t[p][:w, q * H:(q + 1) * H],
                                    op0=ALU.mult, op1=ALU.add,
                                    accum_out=s1_g[:w, 2 * ug + lt:
                                                   2 * ug + lt + 1])
                    for p in pairs:
                        for iu in range(2):
                            ug = p + iu
                            x1in = x1in_t[ug]
                            for lt in range(NK_(ug)):
                                w = CW_(ug, lt)
                                scr = sqp.tile([128, H], BF16, tag="scr")
                                nc.vector.tensor_tensor_reduce(
                                    out=scr[:w],
                                    in0=x1in[:w, lt * H:(lt + 1) * H],
                                    in1=x1in[:w, lt * H:(lt + 1) * H],
                                    scale=1.0, scalar=0.0,
                                    op0=ALU.mult, op1=ALU.add,
                                    accum_out=q1_g[:w, 2 * ug + lt:
                                                   2 * ug + lt + 1])

                mean1, rstd1 = ln_stats(s1_g, q1_g, 2 * GRP)

                # ---------- B2, stage-major per 2-pair block ----------
                for blk in range(GRP // 4):
                    b0 = blk * 4
                    bpairs = (b0, b0 + 2)
                    x1_t, f1_t = {}, {}
                    for p in bpairs:
                        x1 = x12p.tile([128, 512], BF16, tag="x1",
                                       name=f"x1_{g}_{p}")
                        for iu in range(2):
                            ug = p + iu
                            for lt in range(NK_(ug)):
                                w = CW_(ug, lt)
                                q = NK_(p) * iu + lt
                                nc.vector.tensor_scalar(
                                    out=x1[:w, q * H:(q + 1) * H],
                                    in0=x1in_t[ug][:w, lt * H:(lt + 1) * H],
                                    scalar1=mean1[:w, 2 * ug + lt:
                                                  2 * ug + lt + 1],
                                    scalar2=rstd1[:w, 2 * ug + lt:
                                                  2 * ug + lt + 1],
                                    op0=ALU.subtract, op1=ALU.mult)
                        x1_t[p] = x1
                    cblk = sum(ncols[g * GRP + b0 + j] for j in range(4))
                    x1t_ps = psT.tile([H, 1024], BF16, tag="psT")
                    run = 0
                    f1off = {}
                    for p in bpairs:
                        f1off[p] = run
                        for iu in range(2):
                            ug = p + iu
                            for lt in range(NK_(ug)):
                                w = CW_(ug, lt)
                                q = NK_(p) * iu + lt
                                nc.tensor.transpose(
                                    x1t_ps[:, run:run + w],
                                    x1_t[p][:w, q * H:(q + 1) * H],
                                    ident_b[:w, :w])
                                run += w
                    x1T = xTp.tile([H, 1024], BF16, tag="x1T")
                    evac(EV['x1T'], x1T[:, :cblk], x1t_ps[:, :cblk])
                    for p in bpairs:
                        cp = NC_(p) + NC_(p + 1)
                        f1_ps = psB.tile([H, 512], F32, tag="psB")
                        nc.tensor.matmul(f1_ps[:, :cp], w_f1,
                                         x1T[:, f1off[p]:f1off[p] + cp],
                                         start=True, stop=True)
                        f1 = work.tile([H, 512], BF16, tag="f1",
                                       name=f"f1_{g}_{p}")
                        evac(EV['f1'], f1[:, :cp], f1_ps[:, :cp], relu=True)
                        f1_t[p] = f1
                    for p in bpairs:
                        f2_ps = natps.tile([128, 512], F32, tag="natps")
                        for iu in range(2):
                            ug = p + iu
                            for lt in range(NK_(ug)):
                                w = CW_(ug, lt)
                                q = NK_(p) * iu + lt
                                nc.tensor.matmul(
                                    f2_ps[:w, q * H:(q + 1) * H],
                                    f1_t[p][:, NC_(p) * iu + lt * 128:
                                            NC_(p) * iu + lt * 128 + w],
                                    w_f2, start=True, stop=True)
                        for iu in range(2):
                            ug = p + iu
                            for lt in range(NK_(ug)):
                                w = CW_(ug, lt)
                                q = NK_(p) * iu + lt
                                nc.vector.tensor_tensor_reduce(
                                    out=x2in_t[ug][:w, lt * H:(lt + 1) * H],
                                    in0=f2_ps[:w, q * H:(q + 1) * H],
                                    in1=x1_t[p][:w, q * H:(q + 1) * H],
                                    scale=1.0, scalar=0.0,
                                    op0=ALU.add, op1=ALU.add,
                                    accum_out=s2_g[:w, 2 * ug + lt:
                                                   2 * ug + lt + 1])
                    for p in bpairs:
                        for iu in range(2):
                            ug = p + iu
                            for lt in range(NK_(ug)):
                                w = CW_(ug, lt)
                                scr = sqp.tile([128, H], BF16, tag="scr")
                                nc.vector.tensor_tensor_reduce(
                                    out=scr[:w],
                                    in0=x2in_t[ug][:w, lt * H:(lt + 1) * H],
                                    in1=x2in_t[ug][:w, lt * H:(lt + 1) * H],
                                    scale=1.0, scalar=0.0,
                                    op0=ALU.mult, op1=ALU.add,
                                    accum_out=q2_g[:w, 2 * ug + lt:
                                                   2 * ug + lt + 1])

                mcols = m01_all[:, 2 * g * GRP:2 * (g + 1) * GRP]
                mean2, rstd2m = ln_stats(s2_g, q2_g, 2 * GRP, mask_cols=mcols)

                # ---------- B3 ----------
                pool_g = natps.tile([128, 512], F32, tag="natps")
                for p in range(0, GRP, 2):
                    x2 = x12p.tile([128, 512], BF16, tag="x2",
                                   name=f"x2_{g}_{p}")
                    for iu in range(2):
                        ug = p + iu
                        for lt in range(NK_(ug)):
                            w = CW_(ug, lt)
                            q = NK_(p) * iu + lt
                            nc.vector.tensor_scalar(
                                out=x2[:w, q * H:(q + 1) * H],
                                in0=x2in_t[ug][:w, lt * H:(lt + 1) * H],
                                scalar1=mean2[:w, 2 * ug + lt:2 * ug + lt + 1],
                                scalar2=rstd2m[:w, 2 * ug + lt:
                                               2 * ug + lt + 1],
                                op0=ALU.subtract, op1=ALU.mult)
                    for iu in range(2):
                        ug = p + iu
                        for lt in range(NK_(ug)):
                            w = CW_(ug, lt)
                            q = NK_(p) * iu + lt
                            nc.tensor.matmul(
                                pool_g[:H, ug:ug + 1],
                                x2[:w, q * H:(q + 1) * H],
                                ones_b[:w], start=(lt == 0),
                                stop=(lt == NK_(ug) - 1))
                nc.vector.tensor_copy(pooled[:, g * GRP:(g + 1) * GRP],
                                      pool_g[:H, :GRP])

            # ---- per-core tail: unit_fc, building-sum, fusion MLP ----
            u16_ps = natps.tile([128, 512], F32, tag="natps")
            nc.tensor.matmul(u16_ps[:UNITD, :NU], w_u, pooled,
                             start=True, stop=True)
            u16 = work.tile([UNITD, NU], F32, tag="u16")
            nc.scalar.activation(out=u16, in_=u16_ps[:UNITD, :NU],
                                 func=AF.Relu, bias=0.0, scale=1.0)

            u16t_ps = psB.tile([H, 512], F32, tag="psB")
            nc.tensor.transpose(u16t_ps[:NU, :UNITD], u16,
                                ident[:UNITD, :UNITD])
            u16t = work.tile([NU, UNITD], BF16, tag="u16t")
            nc.vector.tensor_copy(u16t, u16t_ps[:NU, :UNITD])

            seq_ps = natps.tile([128, 512], F32, tag="natps")
            nc.tensor.matmul(seq_ps[:UNITD, :BPC], u16t, s_sb,
                             start=True, stop=True)

            fused = work.tile([UNITD + AGGD + TODD, BPC], BF16, tag="fused")
            nc.vector.tensor_copy(fused[:UNITD, :], seq_ps[:UNITD, :BPC])
            nc.gpsimd.dma_start(out=fused[UNITD:, :], in_=tail_in[:, :])

            h1_ps = psB.tile([H, 512], F32, tag="psB")
            nc.tensor.matmul(h1_ps[:H, :BPC], w_c1, fused,
                             start=True, stop=True)
            h1 = work.tile([H, BPC], BF16, tag="h1")
            nc.scalar.activation(out=h1, in_=h1_ps[:H, :BPC], func=AF.Relu,
                                 bias=0.0, scale=1.0)

            o_ps = natps.tile([128, 512], F32, tag="natps")
            nc.tensor.matmul(o_ps[:DOUT, :BPC], w_c2, h1,
                             start=True, stop=True)
            o_s = work.tile([DOUT, BPC], F32, tag="osb")
            nc.scalar.activation(out=o_s, in_=o_ps[:DOUT, :BPC], func=AF.Relu,
                                 bias=0.0, scale=1.0)
            nc.sync.dma_start(out=out_t[:, :], in_=o_s)

    return nc


def _prep_weights(inputs):
    ipw = np.asarray(inputs["in_proj_w"])
    wts = {
        "w_inT": np.asarray(inputs["W_in"]).T,       # [5,128]
        "w_gT": (ipw[0:H] @ ipw[H:2 * H].T),          # Wq^T Wk composed [128,128]
        "w_vT": ipw[2 * H:3 * H].T,
        "w_oT": np.asarray(inputs["out_proj_w"]).T,
        "w_f1T": np.asarray(inputs["W_ff1"]).T,
        "w_f2T": np.asarray(inputs["W_ff2"]).T,
        "w_uT": np.asarray(inputs["W_unit"]).T,       # [128,16]
        "w_c1T": np.asarray(inputs["W_fc1"]).T,       # [26,128]
        "w_c2T": np.asarray(inputs["W_fc2"]).T,       # [128,128]
    }
    wts = {k: np.ascontiguousarray(v.astype(NPBF)) for k, v in wts.items()}
    # the kernel folds no biases / LN affines: assert they are trivial
    for nm in ("b_in", "in_proj_b", "out_proj_b", "b_ff1", "b_ff2",
               "ln1_b", "ln2_b", "b_unit", "b_fc1", "b_fc2"):
        assert np.max(np.abs(np.asarray(inputs[nm]))) == 0.0, f"{nm} nonzero"
    for nm in ("ln1_w", "ln2_w"):
        assert np.allclose(np.asarray(inputs[nm]), 1.0), f"{nm} nontrivial"
    return wts


def make_in_maps(inputs, slens=None):
    x_seq = np.asarray(inputs["x_seq"], dtype=np.float32)       # [B,U,L,5]
    lengths = np.asarray(inputs["lengths"])                      # [B,U] int
    x_agg = np.asarray(inputs["x_agg_quant"], dtype=np.float32)  # [B,7]
    tod_emb = np.asarray(inputs["tod_emb"], dtype=np.float32)    # [5,3]
    tod_idx = np.asarray(inputs["tod_idx"])                      # [B] int

    if slens is None:
        slens = _slens_from_lengths(lengths)
    ncols, nck, ck, go = _sched(slens)
    iota = np.arange(L, dtype=np.float32).reshape(2, 128).T      # [128p, 2]

    in_maps = []
    for c in range(NCORES):
        bs = slice(c * BPC, (c + 1) * BPC)
        lc = lengths[bs].reshape(NU)
        perm = np.argsort(-lc, kind="stable")                    # desc
        lens = lc[perm].astype(np.float32)
        xcT = x_seq[bs].reshape(NU, L, DSEQ)[perm].transpose(0, 2, 1)
        xg = np.zeros((NGRP, DSEQ, GRP * L), np.float32)
        for g in range(NGRP):
            for i in range(GRP):
                s = g * GRP + i
                xg[g, :, go[g][i]:go[g][i] + ncols[s]] = \
                    xcT[s][:, :ncols[s]]
        m01 = (iota[:, None, :] < lens[None, :, None]).astype(np.float32)
        m01 = np.ascontiguousarray(m01.reshape(128, NU * 2))
        eb = (1.0 - m01) * NEGB                                  # 0 / -30
        S = np.zeros((NU, BPC), np.float32)
        S[np.arange(NU), perm // U] = 1.0
        tail = np.concatenate(
            [x_agg[bs].T, tod_emb[tod_idx[bs]].T], axis=0)
        in_maps.append({"xg": np.ascontiguousarray(xg).astype(NPBF),
                        "m01": m01,
                        "eb": np.ascontiguousarray(eb),
                        "S": S.astype(NPBF),
                        "tail": np.ascontiguousarray(tail).astype(NPBF)})
    return in_maps


def kernel(_trace=False, **inputs):
    wts = _prep_weights(inputs)
    slens = _slens_from_lengths(inputs["lengths"])
    nc = build_nc(wts, slens)
    if not nc.is_finalized():
        nc.finalize()
    in_maps = make_in_maps(inputs, slens)
    res = run_bass_kernel_spmd(nc, in_maps, core_ids=list(range(NCORES)),
                               trace=_trace)
    out = np.zeros((B, DOUT), np.float32)
    for c in range(NCORES):
        out[c * BPC:(c + 1) * BPC, :] = res.results[c]["outT"].T
    if _trace:
        kernel._last_results = res
    return out
